# revision 18
# baseline (speedup 1.0000x reference)
"""Trainium2 Bass kernel for nn_ASGSCriterion (SUL focal loss + CEC InfoNCE).

Strategy (data-parallel over batch, 4 images / core on 8 cores):

The reference's [B, C, K_B] boundary-mining structure collapses to per-matched-row
work: matched row n is a valid boundary sample iff its prototype-distance ranks
in the top K_B=5 within its class (strictly-greater count < 5), and the focal
loss of slot (c,k) equals the per-row focal loss of the selected row (single-row
selection commutes with normalization).  Per image:

  obj_n   = normalize(obj)                           [900, 256]
  matched = gather(obj, idx)   (indirect DMA)        [300, 256]
  sims    = matched_n @ protos.T                     [300, 90]   (pos, dists)
  simQ    = matched_n @ obj_n.T  (matched cols of obj_n zeroed)  [300, 900]
  thr     = max(5th-largest(simQ row), tiny)  -> multihot = simQ >= thr
  nbr     = multihot @ obj  (matmul gather-sum)      [256, 300]
  logits  = ((matchedT + nbrT) * 1/(1+wcnt)) @ W.T + b
  fl      = focal loss per row;  sel = rank-in-class < 5;  has = wcnt > 0
  SUL     = sum(sel*has*fl) / max(sum(sel*has), 1)          (global all-reduce)
  CEC     = InfoNCE with fixed logsumexp shift of 10 (S = cos/tau <= 10)

Two tiny AllReduces: [sumexp(90) | sul_num | sul_cnt] mid-kernel, then cec_sum.
"""

import sys

if "/opt/trn_rl_repo" not in sys.path:
    sys.path.insert(0, "/opt/trn_rl_repo")

import numpy as np

import concourse.bass as bass
import concourse.mybir as mybir
import concourse.tile as tile
from concourse import bass_utils
from concourse import library_config

F32 = mybir.dt.float32
F32R = mybir.dt.float32r
I32 = mybir.dt.int32
AF = mybir.ActivationFunctionType
OP = mybir.AluOpType

B, Q, D, Nm, C, NC = 32, 900, 256, 300, 90, 91
NCORES = 8
BL = B // NCORES          # images per core
TAU = 0.1
SHIFT = 10.0              # fixed logsumexp shift; |S| <= 1/TAU = 10
NQT = 8                   # q tiles (900 -> 7*128 + 4)
NMT = 3                   # n tiles (300 -> 2*128 + 44)
QROWS = [128] * 7 + [4]
NROWS = [128, 128, 44]

# dtype knobs for the big matmuls: F32 (exact, 4 cy/row) or F32R (1 cy/row)
CFG = {
    "simq": F32,
    "nbr": F32,
    "logits": F32,
    "mm_small": F32,   # sims/colsums stay fp32
}


def _mmcast(ap, dt):
    return ap.bitcast(dt) if dt != F32 else ap


# ---------------------------------------------------------------------------
# The nix walrus in this container only accepts a small number of sync-wait
# commands per instruction; newer Tile emits up to ~27 on the tail drain and
# 3-5 on some body instructions.  Split excess waits onto preceding same-
# engine NoOps.
# ---------------------------------------------------------------------------
WAIT_LIMIT = 1
_wsplit_n = [0]
_PATCHED = [False]


def _patch_tile_wait_limits():
    if _PATCHED[0]:
        return
    _PATCHED[0] = True
    import bass_rust
    from concourse.vector_clock import ScopedClock

    orig_add = tile.TileContext._add_instruction

    def _make_nop(nc_obj, engine, waits):
        nop = bass_rust.InstNoOp(name=f"I-wsplit{_wsplit_n[0]}", ins=[], outs=[])
        _wsplit_n[0] += 1
        nop.engine = engine
        nop.sync_info = mybir.SyncInfo(on_wait=list(waits), on_update=[])
        return nop

    def patched_add(self, inst):
        si = inst.sync_info
        if si is not None and si.on_wait is not None and len(si.on_wait) > WAIT_LIMIT:
            waits = list(si.on_wait)
            head, keep = waits[:-WAIT_LIMIT], waits[-WAIT_LIMIT:]
            for j in range(0, len(head), WAIT_LIMIT):
                orig_add(self, _make_nop(self.nc, inst.engine, head[j:j + WAIT_LIMIT]))
            si.on_wait = keep
        orig_add(self, inst)

    tile.TileContext._add_instruction = patched_add

    def patched_drain(self, tick_clock, wait_clock):
        probe = self.nc.sync.nop()
        wait_clock.add_sem_waits(
            probe.ins, ScopedClock({None: tick_clock.global_clock})
        )
        psi = probe.ins.sync_info
        waits = list(psi.on_wait) if (psi is not None and psi.on_wait) else []
        chunks = [waits[i:i + WAIT_LIMIT] for i in range(0, len(waits), WAIT_LIMIT)]
        if chunks:
            psi.on_wait = chunks[0]
            for ch in chunks[1:]:
                extra = self.nc.sync.nop()
                extra.ins.sync_info = mybir.SyncInfo(on_wait=list(ch), on_update=[])
        self.nc.sync.drain()
        self.nc.all_engine_barrier()
        assert self.sems is not None
        popped = self.nc._tile_sem_poison_stack.pop()
        assert popped is self._sem_poison
        self.nc.clear_and_free_semaphores(list(self.sems.allocated().values()))
        self.nc.all_engine_barrier()

    tile.TileContext._drain_and_barrier = patched_drain


_patch_tile_wait_limits()


def build_nc(cfg=CFG):
    nc = bass.Bass(
        "TRN2",
        target_bir_lowering=False,
        debug=False,
        enable_asserts=False,
        num_devices=NCORES,
    )
    obj_d = nc.dram_tensor("obj", [BL, Q, D], F32, kind="ExternalInput")
    idx_d = nc.dram_tensor("midx", [BL, Nm], I32, kind="ExternalInput")  # pre-adjusted +b*900
    lab_d = nc.dram_tensor("mlab", [BL, Nm], I32, kind="ExternalInput")
    pro_d = nc.dram_tensor("protos", [C, D], F32, kind="ExternalInput")
    w_d = nc.dram_tensor("wcls", [NC, D], F32, kind="ExternalInput")
    b_d = nc.dram_tensor("bcls", [1, NC], F32, kind="ExternalInput")
    id_d = nc.dram_tensor("identc", [128, 128], F32, kind="ExternalInput")
    io90_d = nc.dram_tensor("iota90c", [128, C], F32, kind="ExternalInput")
    qio_d = nc.dram_tensor("qiotac", [128, NQT], F32, kind="ExternalInput")
    out_d = nc.dram_tensor("out", [2], F32, kind="ExternalOutput")

    ar1_in = nc.dram_tensor("ar1_in", [1, 96], F32)
    ar1_out = nc.dram_tensor("ar1_out", [1, 96], F32, addr_space="Shared")
    ar2_in = nc.dram_tensor("ar2_in", [1, 8], F32)
    ar2_out = nc.dram_tensor("ar2_out", [1, 8], F32, addr_space="Shared")
    groups = [list(range(NCORES))]

    obj_flat = obj_d.ap().rearrange("b q d -> (b q) d")

    with tile.TileContext(nc) as tc:
        with (
            tc.tile_pool(name="const", bufs=1) as cp,
            tc.tile_pool(name="big", bufs=2) as bigp,        # obj / objn [128, 2048]
            tc.tile_pool(name="objnT", bufs=2) as otp,       # [128, 1800]
            tc.tile_pool(name="sq", bufs=2) as sqp,          # simq [128,900] x3
            tc.tile_pool(name="mh", bufs=1) as mhp,          # multihot [128,900] x3
            tc.tile_pool(name="mhT", bufs=1) as mhtp,        # [128, 2400]
            tc.tile_pool(name="med", bufs=2) as medp,        # matched & friends
            tc.tile_pool(name="med1", bufs=1) as medp1,      # bcasts, rawT
            tc.tile_pool(name="small", bufs=2) as smp,       # columns / rows
            tc.tile_pool(name="junk", bufs=2) as jkp,        # scratch outputs
            tc.tile_pool(name="acc", bufs=1) as accp,        # persistent accumulators
            tc.tile_pool(name="ps_sq", bufs=2, space="PSUM") as ps_sq,    # [128,900] = 2 banks
            tc.tile_pool(name="ps_nbr", bufs=1, space="PSUM") as ps_nbr,  # [128,300]
            tc.tile_pool(name="ps_sm", bufs=3, space="PSUM") as ps_sm,    # [128,<=300]
        ):
            # alternate psum->sbuf copies between DVE and ACT
            cp_state = [0]

            def col_bcast(dst, col, r, id_sb):
                """dst[128, :r] = col[:r] broadcast across partitions (PE transpose)."""
                pt = ps_sm.tile([128, 300], F32, tag="pst")
                nc.tensor.transpose(
                    out=pt[:, :r], in_=col[:r, :1].to_broadcast([r, 128]),
                    identity=id_sb[:r, :r],
                )
                copy_out(dst, pt[:, :r])

            def copy_out(dst, src):
                cp_state[0] ^= 1
                if cp_state[0]:
                    nc.vector.tensor_copy(dst, src)
                else:
                    nc.scalar.activation(dst, src, AF.Copy)

            # ---------------- constants ----------------
            id_sb = cp.tile([128, 128], F32)
            nc.sync.dma_start(out=id_sb[:, :], in_=id_d.ap()[:, :])
            io90 = cp.tile([128, C], F32)
            nc.sync.dma_start(out=io90[:, :], in_=io90_d.ap()[:, :])
            qio = cp.tile([128, NQT], F32)
            nc.sync.dma_start(out=qio[:, :], in_=qio_d.ap()[:, :])
            ones_col = cp.tile([128, 1], F32)
            nc.vector.memset(ones_col[:, :], 1.0)
            ones_row = cp.tile([1, 128], F32)
            nc.vector.memset(ones_row[:, :], 1.0)
            bcls_sb = cp.tile([1, NC], F32)
            nc.sync.dma_start(out=bcls_sb[:, :], in_=b_d.ap()[:, :])
            nshift_col = cp.tile([128, 1], F32)
            nc.vector.memset(nshift_col[:, :], -SHIFT)

            # prototypes [90, 256] -> proT [128, 180] (two d-halves)
            pro_sb = cp.tile([C, D], F32)
            nc.sync.dma_start(out=pro_sb[:, :], in_=pro_d.ap()[:, :])
            proT = cp.tile([128, 2 * C], F32)
            for h in range(2):
                pt = ps_sm.tile([128, C], F32, tag="pst")
                nc.tensor.transpose(
                    out=pt[:, :], in_=pro_sb[:, h * 128:(h + 1) * 128],
                    identity=id_sb[:C, :C],
                )
                copy_out(proT[:, h * C:(h + 1) * C], pt[:, :])

            # W_cls [91, 256] -> wT [128, 182]
            w_sb = cp.tile([NC, D], F32)
            nc.sync.dma_start(out=w_sb[:, :], in_=w_d.ap()[:, :])
            wT = cp.tile([128, 2 * NC], F32)
            for h in range(2):
                pt = ps_sm.tile([128, NC], F32, tag="pst")
                nc.tensor.transpose(
                    out=pt[:, :], in_=w_sb[:, h * 128:(h + 1) * 128],
                    identity=id_sb[:NC, :NC],
                )
                copy_out(wT[:, h * NC:(h + 1) * NC], pt[:, :])

            # P = protos @ protos.T / TAU, diag masked; lse over rows (symmetric)
            pP = ps_sm.tile([C, C], F32, tag="pst")
            for h in range(2):
                nc.tensor.matmul(
                    out=pP[:, :], lhsT=proT[:, h * C:(h + 1) * C],
                    rhs=proT[:, h * C:(h + 1) * C], start=(h == 0), stop=(h == 1),
                )
            P_sb = cp.tile([C, C], F32)
            # P/TAU - 1e9*I
            idbig = cp.tile([C, C], F32)
            nc.vector.tensor_scalar(
                out=idbig[:, :], in0=id_sb[:C, :C], scalar1=1e9, scalar2=None,
                op0=OP.mult,
            )
            nc.vector.tensor_scalar(
                out=P_sb[:, :], in0=pP[:, :], scalar1=1.0 / TAU, scalar2=None,
                op0=OP.mult,
            )
            nc.vector.tensor_tensor(out=P_sb[:, :], in0=P_sb[:, :], in1=idbig[:, :], op=OP.subtract)
            pmax = cp.tile([C, 1], F32)
            nc.vector.tensor_reduce(out=pmax[:, :], in_=P_sb[:, :], axis=mybir.AxisListType.X, op=OP.max)
            npmax = cp.tile([C, 1], F32)
            nc.vector.tensor_scalar(out=npmax[:, :], in0=pmax[:, :], scalar1=-1.0, scalar2=None, op0=OP.mult)
            pexp = cp.tile([C, C], F32)
            psum_col = cp.tile([C, 1], F32)
            nc.scalar.activation(pexp[:, :], P_sb[:, :], AF.Exp, bias=npmax[:, :1], scale=1.0, accum_out=psum_col[:, :1])
            plog = cp.tile([C, 1], F32)
            nc.scalar.activation(plog[:, :], psum_col[:, :], AF.Ln)
            lsePm_col = cp.tile([C, 1], F32)
            nc.vector.tensor_tensor(out=lsePm_col[:, :], in0=plog[:, :], in1=pmax[:, :], op=OP.add)


            # persistent accumulators: cols 0:90 expnet | 90 sul_num | 91 sul_cnt | 92 cec
            acc = accp.tile([128, 96], F32)
            nc.vector.memset(acc[:, :], 0.0)
            labf_all = accp.tile([128, BL * NMT], F32)
            posc_all = accp.tile([128, BL * NMT], F32)

            # ---------------- phase 1: per image ----------------
            for b in range(BL):
                # ---- loads ----
                obj_sb = bigp.tile([128, NQT * D], F32, tag="obj")
                nc.gpsimd.memset(obj_sb[:, 7 * D:], 0.0)
                nc.sync.dma_start(
                    out=obj_sb[:, :7 * D].rearrange("p (t d) -> p t d", d=D),
                    in_=obj_d.ap()[b, :7 * 128, :].rearrange("(t p) d -> p t d", p=128),
                )
                nc.sync.dma_start(out=obj_sb[:4, 7 * D:], in_=obj_d.ap()[b, 7 * 128:, :])

                idxc = smp.tile([128, NMT], I32, tag="idxc")
                nc.gpsimd.memset(idxc[:, :], 0)
                labc = smp.tile([128, NMT], I32, tag="labc")
                nc.gpsimd.memset(labc[:, :], 1 << 30)
                for m in range(NMT):
                    r = NROWS[m]
                    nc.sync.dma_start(
                        out=idxc[:r, m:m + 1],
                        in_=idx_d.ap()[b, m * 128:m * 128 + r].rearrange("(p o) -> p o", o=1),
                    )
                    nc.sync.dma_start(
                        out=labc[:r, m:m + 1],
                        in_=lab_d.ap()[b, m * 128:m * 128 + r].rearrange("(p o) -> p o", o=1),
                    )
                idxcf = smp.tile([128, NMT], F32, tag="idxcf")
                nc.vector.tensor_copy(idxcf[:, :], idxc[:, :])
                for m in range(NMT):
                    nc.vector.tensor_copy(labf_all[:, b * NMT + m: b * NMT + m + 1], labc[:, m:m + 1])

                # ---- matched gather (indices pre-adjusted by +b*900 host-side) ----
                matched = medp.tile([128, NMT * D], F32, tag="matched")
                for m in range(NMT):
                    r = NROWS[m]
                    nc.gpsimd.indirect_dma_start(
                        out=matched[:r, m * D:(m + 1) * D],
                        out_offset=None,
                        in_=obj_flat[:, :],
                        in_offset=bass.IndirectOffsetOnAxis(ap=idxc[:r, m:m + 1], axis=0),
                    )

                # ---- norms ----
                q2 = smp.tile([128, NQT], F32, tag="q2")
                for t in range(NQT):
                    jt = jkp.tile([128, D], F32, tag="j256")
                    nc.scalar.activation(
                        jt[:, :], obj_sb[:, t * D:(t + 1) * D], AF.Square,
                        accum_out=q2[:, t:t + 1],
                    )
                qn = smp.tile([128, NQT], F32, tag="qn")
                nc.scalar.activation(qn[:, :], q2[:, :], AF.Sqrt)
                nc.vector.tensor_scalar(out=qn[:, :], in0=qn[:, :], scalar1=1e-12, scalar2=None, op0=OP.max)
                rq = smp.tile([128, NQT], F32, tag="rq")
                nc.vector.reciprocal(rq[:, :], qn[:, :])

                # is_matched via compare with broadcast idx row
                idx_bc = medp1.tile([128, Nm], F32, tag="idxbc")
                for m in range(NMT):
                    col_bcast(idx_bc[:, m * 128: m * 128 + NROWS[m]], idxcf[:, m:m + 1], NROWS[m], id_sb)
                qio_b = smp.tile([128, NQT], F32, tag="qiob")
                nc.vector.tensor_scalar(out=qio_b[:, :], in0=qio[:, :], scalar1=float(b * Q), scalar2=None, op0=OP.add)
                ism = smp.tile([128, NQT], F32, tag="ism")
                for t in range(NQT):
                    jt = jkp.tile([128, Nm], F32, tag="j300")
                    nc.vector.tensor_scalar(
                        out=jt[:, :], in0=idx_bc[:, :], scalar1=qio_b[:, t:t + 1],
                        scalar2=None, op0=OP.is_equal, op1=OP.add,
                        accum_out=ism[:, t:t + 1],
                    )
                # rqm = rq * (1 - ism)
                rqm = smp.tile([128, NQT], F32, tag="rqm")
                nc.vector.tensor_scalar(out=rqm[:, :], in0=ism[:, :], scalar1=-1.0, scalar2=1.0, op0=OP.mult, op1=OP.add)
                nc.vector.tensor_tensor(out=rqm[:, :], in0=rqm[:, :], in1=rq[:, :], op=OP.mult)

                objn = bigp.tile([128, NQT * D], F32, tag="objn")
                for t in range(NQT):
                    nc.scalar.activation(
                        objn[:, t * D:(t + 1) * D], obj_sb[:, t * D:(t + 1) * D],
                        AF.Copy, scale=rqm[:, t:t + 1],
                    )

                # matched norms + normalize
                m2 = smp.tile([128, NMT], F32, tag="m2")
                nc.vector.memset(m2[:, :], 1.0)
                for m in range(NMT):
                    r = NROWS[m]
                    jt = jkp.tile([128, D], F32, tag="j256")
                    nc.scalar.activation(
                        jt[:r, :], matched[:r, m * D:(m + 1) * D], AF.Square,
                        accum_out=m2[:r, m:m + 1],
                    )
                mn = smp.tile([128, NMT], F32, tag="mn")
                nc.scalar.activation(mn[:, :], m2[:, :], AF.Sqrt)
                nc.vector.tensor_scalar(out=mn[:, :], in0=mn[:, :], scalar1=1e-12, scalar2=None, op0=OP.max)
                rm = smp.tile([128, NMT], F32, tag="rm")
                nc.vector.reciprocal(rm[:, :], mn[:, :])
                matched_n = medp.tile([128, NMT * D], F32, tag="matchedn")
                for m in range(NMT):
                    r = NROWS[m]
                    nc.scalar.activation(
                        matched_n[:r, m * D:(m + 1) * D], matched[:r, m * D:(m + 1) * D],
                        AF.Copy, scale=rm[:r, m:m + 1],
                    )

                # ---- transposes: matched_nT, matchedT [128, 600], objnT [128, 1800] ----
                mnT = medp.tile([128, 2 * Nm], F32, tag="mnT")
                mT = medp1.tile([128, 2 * Nm], F32, tag="mT")
                for m in range(NMT):
                    r = NROWS[m]
                    for h in range(2):
                        pt = ps_sm.tile([128, 300], F32, tag="pst")
                        nc.tensor.transpose(
                            out=pt[:, :r],
                            in_=matched_n[:r, m * D + h * 128: m * D + (h + 1) * 128],
                            identity=id_sb[:r, :r],
                        )
                        copy_out(mnT[:, h * Nm + m * 128: h * Nm + m * 128 + r], pt[:, :r])
                        pt2 = ps_sm.tile([128, 300], F32, tag="pst")
                        nc.tensor.transpose(
                            out=pt2[:, :r],
                            in_=matched[:r, m * D + h * 128: m * D + (h + 1) * 128],
                            identity=id_sb[:r, :r],
                        )
                        copy_out(mT[:, h * Nm + m * 128: h * Nm + m * 128 + r], pt2[:, :r])

                objnT = otp.tile([128, 2 * Q], F32, tag="objnT")
                for t in range(NQT):
                    r = QROWS[t]
                    for h in range(2):
                        pt = ps_sm.tile([128, 300], F32, tag="pst")
                        nc.tensor.transpose(
                            out=pt[:, :r],
                            in_=objn[:r, t * D + h * 128: t * D + (h + 1) * 128],
                            identity=id_sb[:r, :r],
                        )
                        copy_out(objnT[:, h * Q + t * 128: h * Q + t * 128 + r], pt[:, :r])

                # ---- sims = matched_n @ protos.T  [300, 90] ----
                sims_sb = medp.tile([128, NMT * C], F32, tag="sims")
                for m in range(NMT):
                    r = NROWS[m]
                    psim = ps_sm.tile([128, 300], F32, tag="pst")
                    for h in range(2):
                        nc.tensor.matmul(
                            out=psim[:r, :C],
                            lhsT=mnT[:, h * Nm + m * 128: h * Nm + m * 128 + r],
                            rhs=proT[:, h * C:(h + 1) * C],
                            start=(h == 0), stop=(h == 1),
                        )
                    copy_out(sims_sb[:r, m * C:(m + 1) * C], psim[:r, :C])

                # ---- simQ = matched_n @ obj_n.T  [300, 900] ----
                simq_sb = sqp.tile([128, NMT * Q], F32, tag="simq")
                for m in range(NMT):
                    r = NROWS[m]
                    psq = ps_sq.tile([128, Q], F32, tag="psq")
                    for c0, c1 in ((0, 512), (512, Q)):
                        for h in range(2):
                            nc.tensor.matmul(
                                out=psq[:r, c0:c1],
                                lhsT=_mmcast(mnT[:, h * Nm + m * 128: h * Nm + m * 128 + r], cfg["simq"]),
                                rhs=_mmcast(objnT[:, h * Q + c0: h * Q + c1], cfg["simq"]),
                                start=(h == 0), stop=(h == 1),
                            )
                    copy_out(simq_sb[:r, m * Q:(m + 1) * Q], psq[:r, :])

                # ---- top-5 threshold, multihot, wcnt ----
                mh = mhp.tile([128, NMT * Q], F32, tag="mh")
                wcnt = smp.tile([128, NMT], F32, tag="wcnt")
                nc.vector.memset(wcnt[:, :], 0.0)
                thr = smp.tile([128, NMT], F32, tag="thr")
                for m in range(NMT):
                    r = NROWS[m]
                    mx8 = jkp.tile([128, 8], F32, tag="mx8")
                    nc.vector.max(out=mx8[:r, :], in_=simq_sb[:r, m * Q:(m + 1) * Q])
                    nc.vector.tensor_scalar(out=thr[:r, m:m + 1], in0=mx8[:r, 4:5], scalar1=1e-30, scalar2=None, op0=OP.max)
                    nc.vector.tensor_scalar(
                        out=mh[:r, m * Q:(m + 1) * Q], in0=simq_sb[:r, m * Q:(m + 1) * Q],
                        scalar1=thr[:r, m:m + 1], scalar2=None,
                        op0=OP.is_ge, op1=OP.add, accum_out=wcnt[:r, m:m + 1],
                    )

                # ---- multihot transpose [q, n] ----
                mhT = mhtp.tile([128, NQT * Nm], F32, tag="mhT")
                for m in range(NMT):
                    r = NROWS[m]
                    for t in range(NQT):
                        qr = QROWS[t]
                        pt = ps_sm.tile([128, 300], F32, tag="pst")
                        nc.tensor.transpose(
                            out=pt[:qr, :r],
                            in_=mh[:r, m * Q + t * 128: m * Q + t * 128 + qr],
                            identity=id_sb[:r, :r],
                        )
                        copy_out(mhT[:qr, t * Nm + m * 128: t * Nm + m * 128 + r], pt[:qr, :r])

                # ---- nbr sum: rawT = matchedT + obj.T @ multihot.T  [256 x 300] ----
                rawT = medp1.tile([128, 2 * Nm], F32, tag="rawT")
                for h in range(2):
                    pn = ps_nbr.tile([128, Nm], F32, tag="pnbr")
                    for t in range(NQT):
                        qr = QROWS[t]
                        nc.tensor.matmul(
                            out=pn[:, :],
                            lhsT=_mmcast(obj_sb[:qr, t * D + h * 128: t * D + (h + 1) * 128], cfg["nbr"]),
                            rhs=_mmcast(mhT[:qr, t * Nm:(t + 1) * Nm], cfg["nbr"]),
                            start=(t == 0), stop=(t == NQT - 1),
                        )
                    nc.vector.tensor_tensor(
                        out=rawT[:, h * Nm:(h + 1) * Nm], in0=pn[:, :],
                        in1=mT[:, h * Nm:(h + 1) * Nm], op=OP.add,
                    )

                # ---- logits & focal ----
                den = smp.tile([128, NMT], F32, tag="den")
                nc.vector.tensor_scalar(out=den[:, :], in0=wcnt[:, :], scalar1=1.0, scalar2=None, op0=OP.add)
                sden = smp.tile([128, NMT], F32, tag="sden")
                nc.vector.reciprocal(sden[:, :], den[:, :])

                fl = smp.tile([128, NMT], F32, tag="fl")
                hasn = smp.tile([128, NMT], F32, tag="hasn")
                nc.vector.tensor_scalar(out=hasn[:, :], in0=wcnt[:, :], scalar1=0.5, scalar2=None, op0=OP.is_gt)

                for m in range(NMT):
                    r = NROWS[m]
                    pl = ps_sm.tile([128, 300], F32, tag="pst")
                    for h in range(2):
                        nc.tensor.matmul(
                            out=pl[:r, :NC],
                            lhsT=_mmcast(rawT[:, h * Nm + m * 128: h * Nm + m * 128 + r], cfg["logits"]),
                            rhs=_mmcast(wT[:, h * NC:(h + 1) * NC], cfg["logits"]),
                            start=(h == 0), stop=False,
                        )
                    nc.tensor.matmul(
                        out=pl[:r, :NC], lhsT=ones_row[:1, :r], rhs=bcls_sb[:1, :],
                        start=False, stop=True,
                    )
                    lg = jkp.tile([128, NC], F32, tag="lg")
                    nc.vector.tensor_scalar(out=lg[:r, :], in0=pl[:r, :NC], scalar1=sden[:r, m:m + 1], scalar2=None, op0=OP.mult)
                    # focal: X_j = softplus(l)*sig(l)^2 ; last col uses -l
                    sg = jkp.tile([128, NC], F32, tag="sg")
                    nc.scalar.activation(sg[:r, :], lg[:r, :], AF.Sigmoid)
                    sp = jkp.tile([128, NC], F32, tag="sp")
                    nc.scalar.activation(sp[:r, :], lg[:r, :], AF.Exp)
                    nc.scalar.activation(sp[:r, :], sp[:r, :], AF.Ln, bias=1.0, scale=1.0)
                    s2 = jkp.tile([128, NC], F32, tag="s2")
                    nc.vector.tensor_tensor(out=s2[:r, :], in0=sg[:r, :], in1=sg[:r, :], op=OP.mult)
                    X = jkp.tile([128, NC], F32, tag="X")
                    xs = jkp.tile([128, 1], F32, tag="xs")
                    nc.vector.tensor_tensor(out=X[:r, :], in0=s2[:r, :], in1=sp[:r, :], op=OP.mult)
                    nc.vector.tensor_reduce(out=xs[:r, :1], in_=X[:r, :], axis=mybir.AxisListType.X, op=OP.add)
                    # Y = softplus(-l_last)*sig(-l_last)^2
                    nl = jkp.tile([128, 1], F32, tag="nl")
                    nc.vector.tensor_scalar(out=nl[:r, :], in0=lg[:r, NC - 1:NC], scalar1=-1.0, scalar2=None, op0=OP.mult)
                    sgn = jkp.tile([128, 1], F32, tag="sgn")
                    nc.scalar.activation(sgn[:r, :], nl[:r, :], AF.Sigmoid)
                    spn = jkp.tile([128, 1], F32, tag="spn")
                    nc.scalar.activation(spn[:r, :], nl[:r, :], AF.Exp)
                    nc.scalar.activation(spn[:r, :], spn[:r, :], AF.Ln, bias=1.0, scale=1.0)
                    Y = jkp.tile([128, 1], F32, tag="Y")
                    nc.vector.tensor_tensor(out=Y[:r, :], in0=sgn[:r, :], in1=sgn[:r, :], op=OP.mult)
                    nc.vector.tensor_tensor(out=Y[:r, :], in0=Y[:r, :], in1=spn[:r, :], op=OP.mult)
                    # fl = (0.75*(xs - X_last) + 0.25*Y)/NC
                    t1 = jkp.tile([128, 1], F32, tag="t1")
                    nc.vector.tensor_tensor(out=t1[:r, :], in0=xs[:r, :], in1=X[:r, NC - 1:NC], op=OP.subtract)
                    nc.vector.tensor_scalar(out=t1[:r, :], in0=t1[:r, :], scalar1=0.75 / NC, scalar2=None, op0=OP.mult)
                    nc.vector.tensor_scalar(out=Y[:r, :], in0=Y[:r, :], scalar1=0.25 / NC, scalar2=None, op0=OP.mult)
                    nc.vector.tensor_tensor(out=fl[:r, m:m + 1], in0=t1[:r, :], in1=Y[:r, :], op=OP.add)

                # ---- pos, dists, CEC exp accumulation, rank-in-class ----
                dcol = smp.tile([128, NMT], F32, tag="dcol")
                for m in range(NMT):
                    r = NROWS[m]
                    mask = jkp.tile([128, C], F32, tag="mask")
                    nc.vector.tensor_scalar(
                        out=mask[:r, :], in0=io90[:r, :], scalar1=labf_all[:r, b * NMT + m: b * NMT + m + 1],
                        scalar2=None, op0=OP.is_equal,
                    )
                    j90 = jkp.tile([128, C], F32, tag="j90")
                    nc.vector.tensor_tensor(out=j90[:r, :], in0=sims_sb[:r, m * C:(m + 1) * C], in1=mask[:r, :], op=OP.mult)
                    nc.vector.tensor_reduce(out=posc_all[:r, b * NMT + m: b * NMT + m + 1], in_=j90[:r, :], axis=mybir.AxisListType.X, op=OP.add)
                    nc.vector.tensor_scalar(
                        out=dcol[:r, m:m + 1], in0=posc_all[:r, b * NMT + m: b * NMT + m + 1],
                        scalar1=-1.0, scalar2=1.0, op0=OP.mult, op1=OP.add,
                    )
                    # expnet += exp(10*sims - 10) * (1 - mask)
                    expm = jkp.tile([128, C], F32, tag="expm")
                    nc.scalar.activation(expm[:r, :], sims_sb[:r, m * C:(m + 1) * C], AF.Exp, bias=nshift_col[:r, :1], scale=1.0 / TAU)
                    nm_ = jkp.tile([128, C], F32, tag="nm_")
                    nc.vector.tensor_scalar(out=nm_[:r, :], in0=mask[:r, :], scalar1=-1.0, scalar2=1.0, op0=OP.mult, op1=OP.add)
                    nc.vector.tensor_tensor(out=expm[:r, :], in0=expm[:r, :], in1=nm_[:r, :], op=OP.mult)
                    nc.vector.tensor_tensor(out=acc[:r, 0:C], in0=acc[:r, 0:C], in1=expm[:r, :], op=OP.add)

                d_bc = medp1.tile([128, Nm], F32, tag="dbc")
                lab_bc = medp1.tile([128, Nm], F32, tag="labbc")
                for m in range(NMT):
                    r = NROWS[m]
                    col_bcast(d_bc[:, m * 128: m * 128 + r], dcol[:, m:m + 1], r, id_sb)
                    col_bcast(lab_bc[:, m * 128: m * 128 + r],
                              labf_all[:, b * NMT + m: b * NMT + m + 1], r, id_sb)

                for m in range(NMT):
                    r = NROWS[m]
                    eq = jkp.tile([128, Nm], F32, tag="eq")
                    nc.vector.tensor_scalar(
                        out=eq[:r, :], in0=lab_bc[:r, :],
                        scalar1=labf_all[:r, b * NMT + m: b * NMT + m + 1], scalar2=None, op0=OP.is_equal,
                    )
                    gt = jkp.tile([128, Nm], F32, tag="gt")
                    nc.vector.tensor_scalar(
                        out=gt[:r, :], in0=d_bc[:r, :], scalar1=dcol[:r, m:m + 1],
                        scalar2=None, op0=OP.is_gt,
                    )
                    j300 = jkp.tile([128, Nm], F32, tag="j300b")
                    cnt = jkp.tile([128, 1], F32, tag="cnt")
                    nc.vector.tensor_tensor(out=j300[:r, :], in0=eq[:r, :], in1=gt[:r, :], op=OP.mult)
                    nc.vector.tensor_reduce(out=cnt[:r, :1], in_=j300[:r, :], axis=mybir.AxisListType.X, op=OP.add)
                    sel = jkp.tile([128, 1], F32, tag="sel")
                    nc.vector.tensor_scalar(out=sel[:r, :], in0=cnt[:r, :], scalar1=4.5, scalar2=None, op0=OP.is_lt)
                    c1 = jkp.tile([128, 1], F32, tag="c1")
                    nc.vector.tensor_tensor(out=c1[:r, :], in0=sel[:r, :], in1=hasn[:r, m:m + 1], op=OP.mult)
                    c2 = jkp.tile([128, 1], F32, tag="c2")
                    nc.vector.tensor_tensor(out=c2[:r, :], in0=c1[:r, :], in1=fl[:r, m:m + 1], op=OP.mult)
                    nc.vector.tensor_tensor(out=acc[:r, C:C + 1], in0=acc[:r, C:C + 1], in1=c2[:r, :], op=OP.add)
                    nc.vector.tensor_tensor(out=acc[:r, C + 1:C + 2], in0=acc[:r, C + 1:C + 2], in1=c1[:r, :], op=OP.add)

            # ---------------- AllReduce 1: [expnet(90) | sul_num | sul_cnt] ----------------
            pr1 = ps_sm.tile([1, 300], F32, tag="pst")
            nc.tensor.matmul(out=pr1[:1, :92], lhsT=ones_col[:, :1], rhs=acc[:, 0:92], start=True, stop=True)
            r1 = smp.tile([1, 96], F32, tag="r1")
            nc.vector.memset(r1[:, :], 0.0)
            nc.vector.tensor_copy(r1[:1, :92], pr1[:1, :92])
            nc.sync.dma_start(out=ar1_in.ap()[:, :], in_=r1[:, :])
            nc.gpsimd.collective_compute(
                "AllReduce", OP.add, replica_groups=groups,
                ins=[ar1_in.ap()[:, :]], outs=[ar1_out.ap()[:, :]],
            )
            g1 = smp.tile([96, 1], F32, tag="g1")
            nc.sync.dma_start(out=g1[:, :], in_=ar1_out.ap()[0, :].rearrange("(p o) -> p o", o=1))

            # lse_neg col = logaddexp(lsePm, SHIFT + ln(sumexp))
            lnS = smp.tile([C, 1], F32, tag="lnS")
            nc.scalar.activation(lnS[:, :], g1[:C, :], AF.Ln)
            nc.vector.tensor_scalar(out=lnS[:, :], in0=lnS[:, :], scalar1=SHIFT, scalar2=None, op0=OP.add)
            mx = smp.tile([C, 1], F32, tag="mx")
            nc.vector.tensor_tensor(out=mx[:, :], in0=lnS[:, :], in1=lsePm_col[:, :], op=OP.max)
            mnm = smp.tile([C, 1], F32, tag="mnm")
            nc.vector.tensor_tensor(out=mnm[:, :], in0=lnS[:, :], in1=lsePm_col[:, :], op=OP.min)
            nc.vector.tensor_tensor(out=mnm[:, :], in0=mnm[:, :], in1=mx[:, :], op=OP.subtract)
            ef = smp.tile([C, 1], F32, tag="ef")
            nc.scalar.activation(ef[:, :], mnm[:, :], AF.Exp)
            l1 = smp.tile([C, 1], F32, tag="l1")
            nc.scalar.activation(l1[:, :], ef[:, :], AF.Ln, bias=1.0, scale=1.0)
            lneg = smp.tile([C, 1], F32, tag="lneg")
            nc.vector.tensor_tensor(out=lneg[:, :], in0=mx[:, :], in1=l1[:, :], op=OP.add)
            ln_bc = medp1.tile([128, C], F32, tag="lnbc")
            col_bcast(ln_bc[:, :C], lneg[:, :1], C, id_sb)

            # ---------------- phase 2: per-sample CEC ----------------
            for b in range(BL):
                for m in range(NMT):
                    r = NROWS[m]
                    k = b * NMT + m
                    mask = jkp.tile([128, C], F32, tag="mask")
                    nc.vector.tensor_scalar(
                        out=mask[:r, :], in0=io90[:r, :], scalar1=labf_all[:r, k:k + 1],
                        scalar2=None, op0=OP.is_equal,
                    )
                    j90 = jkp.tile([128, C], F32, tag="j90")
                    lnn = jkp.tile([128, 1], F32, tag="lnn")
                    nc.vector.tensor_tensor(out=j90[:r, :], in0=ln_bc[:r, :], in1=mask[:r, :], op=OP.mult)
                    nc.vector.tensor_reduce(out=lnn[:r, :1], in_=j90[:r, :], axis=mybir.AxisListType.X, op=OP.add)
                    posS = jkp.tile([128, 1], F32, tag="posS")
                    nc.vector.tensor_scalar(out=posS[:r, :], in0=posc_all[:r, k:k + 1], scalar1=1.0 / TAU, scalar2=None, op0=OP.mult)
                    mxc = jkp.tile([128, 1], F32, tag="mxc")
                    nc.vector.tensor_tensor(out=mxc[:r, :], in0=posS[:r, :], in1=lnn[:r, :], op=OP.max)
                    mnc = jkp.tile([128, 1], F32, tag="mnc")
                    nc.vector.tensor_tensor(out=mnc[:r, :], in0=posS[:r, :], in1=lnn[:r, :], op=OP.min)
                    nc.vector.tensor_tensor(out=mnc[:r, :], in0=mnc[:r, :], in1=mxc[:r, :], op=OP.subtract)
                    efc = jkp.tile([128, 1], F32, tag="efc")
                    nc.scalar.activation(efc[:r, :], mnc[:r, :], AF.Exp)
                    l1c = jkp.tile([128, 1], F32, tag="l1c")
                    nc.scalar.activation(l1c[:r, :], efc[:r, :], AF.Ln, bias=1.0, scale=1.0)
                    nc.vector.tensor_tensor(out=mxc[:r, :], in0=mxc[:r, :], in1=l1c[:r, :], op=OP.add)
                    nc.vector.tensor_tensor(out=mxc[:r, :], in0=mxc[:r, :], in1=posS[:r, :], op=OP.subtract)
                    nc.vector.tensor_tensor(out=acc[:r, 92:93], in0=acc[:r, 92:93], in1=mxc[:r, :], op=OP.add)

            # ---------------- AllReduce 2: cec_sum ----------------
            pr2 = ps_sm.tile([1, 300], F32, tag="pst")
            nc.tensor.matmul(out=pr2[:1, :1], lhsT=ones_col[:, :1], rhs=acc[:, 92:93], start=True, stop=True)
            r2 = smp.tile([1, 8], F32, tag="r2")
            nc.vector.memset(r2[:, :], 0.0)
            nc.vector.tensor_copy(r2[:1, :1], pr2[:1, :1])
            nc.sync.dma_start(out=ar2_in.ap()[:, :], in_=r2[:, :])
            nc.gpsimd.collective_compute(
                "AllReduce", OP.add, replica_groups=groups,
                ins=[ar2_in.ap()[:, :]], outs=[ar2_out.ap()[:, :]],
            )
            g2 = smp.tile([1, 8], F32, tag="g2")
            nc.sync.dma_start(out=g2[:, :], in_=ar2_out.ap()[:, :])

            # ---------------- final output ----------------
            outr = smp.tile([1, 2], F32, tag="outr")
            sulrow = smp.tile([1, 2], F32, tag="sulrow")
            nc.sync.dma_start(out=sulrow[:, :], in_=ar1_out.ap()[:, 90:92])
            denf = smp.tile([1, 1], F32, tag="denf")
            nc.vector.tensor_scalar(out=denf[:, :], in0=sulrow[:1, 1:2], scalar1=1.0, scalar2=None, op0=OP.max)
            rdf = smp.tile([1, 1], F32, tag="rdf")
            nc.vector.reciprocal(rdf[:, :], denf[:, :])
            nc.vector.tensor_tensor(out=outr[:1, 0:1], in0=sulrow[:1, 0:1], in1=rdf[:1, :], op=OP.mult)
            nc.vector.tensor_scalar(out=outr[:1, 1:2], in0=g2[:1, 0:1], scalar1=1.0 / (B * Nm), scalar2=None, op0=OP.mult)
            nc.sync.dma_start(out=out_d.ap().rearrange("(a b) -> a b", a=1), in_=outr[:, :])

    return nc


def make_in_maps(obj_embs, prototypes, W_cls, b_cls, match_src_idx, match_labels):
    identc = np.eye(128, dtype=np.float32)
    iota90c = np.tile(np.arange(C, dtype=np.float32), (128, 1))
    qiotac = (np.arange(128, dtype=np.float32)[:, None]
              + 128.0 * np.arange(NQT, dtype=np.float32)[None, :]).astype(np.float32)
    adj = (np.arange(BL, dtype=np.int32) * Q)[:, None]
    in_maps = []
    for c in range(NCORES):
        sl = slice(c * BL, (c + 1) * BL)
        in_maps.append({
            "obj": np.ascontiguousarray(obj_embs[sl]).astype(np.float32),
            "midx": (match_src_idx[sl].astype(np.int32) + adj),
            "mlab": np.ascontiguousarray(match_labels[sl]).astype(np.int32),
            "protos": np.ascontiguousarray(prototypes).astype(np.float32),
            "wcls": np.ascontiguousarray(W_cls).astype(np.float32),
            "bcls": np.ascontiguousarray(b_cls).astype(np.float32).reshape(1, NC),
            "identc": identc,
            "iota90c": iota90c,
            "qiotac": qiotac,
        })
    return in_maps


_CACHE = {}


def _install_ntff_shim():
    """Register the axon NTFF profile hook (test-time only; grading never traces)."""
    import types
    try:
        from antenv.axon_hooks import get_axon_ntff_profile_hook  # noqa: F401
        return
    except ImportError:
        pass
    import antenv
    from trn_agent_boot.trn_boot import _ntff_profile_via_ctypes
    mod = types.ModuleType("antenv.axon_hooks")
    _hook = [None]
    mod.set_axon_ntff_profile_hook = lambda h: _hook.__setitem__(0, h)
    mod.get_axon_ntff_profile_hook = lambda: _hook[0]
    sys.modules["antenv.axon_hooks"] = mod
    antenv.axon_hooks = mod
    mod.set_axon_ntff_profile_hook(_ntff_profile_via_ctypes("/opt/axon/libaxon_pjrt.so"))
    orig_upload = bass_utils.upload_artifacts
    def _safe_upload(tmpdir):
        try:
            return orig_upload(tmpdir)
        except Exception as e:
            print("upload_artifacts skipped:", e)
            return tmpdir
    bass_utils.upload_artifacts = _safe_upload


def kernel(obj_embs, prototypes, W_cls, b_cls, match_src_idx, match_labels,
           _trace=False, **extra):
    if _trace:
        _install_ntff_shim()
    if "nc" not in _CACHE:
        _CACHE["nc"] = build_nc()
    nc = _CACHE["nc"]
    in_maps = make_in_maps(obj_embs, prototypes, W_cls, b_cls,
                           match_src_idx, match_labels)
    res = bass_utils.run_bass_kernel_spmd(
        nc, in_maps, core_ids=list(range(NCORES)), trace=_trace,
    )
    _CACHE["last_results"] = res
    return np.asarray(res.results[0]["out"], dtype=np.float32).reshape(2)


if __name__ == "__main__":
    nc = build_nc()
    print("built ok")


# revision 20
# speedup vs baseline: 1.0716x; 1.0716x over previous
"""Trainium2 Bass kernel for nn_ASGSCriterion (SUL focal loss + CEC InfoNCE).

Strategy (data-parallel over batch, 4 images / core on 8 cores):

The reference's [B, C, K_B] boundary-mining structure collapses to per-matched-row
work: matched row n is a valid boundary sample iff its prototype-distance ranks
in the top K_B=5 within its class (strictly-greater count < 5), and the focal
loss of slot (c,k) equals the per-row focal loss of the selected row (single-row
selection commutes with normalization).  Per image:

  obj_n   = normalize(obj)                           [900, 256]
  matched = gather(obj, idx)   (indirect DMA)        [300, 256]
  sims    = matched_n @ protos.T                     [300, 90]   (pos, dists)
  simQ    = matched_n @ obj_n.T  (matched cols of obj_n zeroed)  [300, 900]
  thr     = max(5th-largest(simQ row), tiny)  -> multihot = simQ >= thr
  nbr     = multihot @ obj  (matmul gather-sum)      [256, 300]
  logits  = ((matchedT + nbrT) * 1/(1+wcnt)) @ W.T + b
  fl      = focal loss per row;  sel = rank-in-class < 5;  has = wcnt > 0
  SUL     = sum(sel*has*fl) / max(sum(sel*has), 1)          (global all-reduce)
  CEC     = InfoNCE with fixed logsumexp shift of 10 (S = cos/tau <= 10)

Two tiny AllReduces: [sumexp(90) | sul_num | sul_cnt] mid-kernel, then cec_sum.
"""

import sys

if "/opt/trn_rl_repo" not in sys.path:
    sys.path.insert(0, "/opt/trn_rl_repo")

import numpy as np

import concourse.bass as bass
import concourse.mybir as mybir
import concourse.tile as tile
from concourse import bass_utils
from concourse import library_config

F32 = mybir.dt.float32
F32R = mybir.dt.float32r
I32 = mybir.dt.int32
AF = mybir.ActivationFunctionType
OP = mybir.AluOpType

B, Q, D, Nm, C, NC = 32, 900, 256, 300, 90, 91
NCORES = 8
BL = B // NCORES          # images per core
TAU = 0.1
SHIFT = 10.0              # fixed logsumexp shift; |S| <= 1/TAU = 10
NQT = 8                   # q tiles (900 -> 7*128 + 4)
NMT = 3                   # n tiles (300 -> 2*128 + 44)
QROWS = [128] * 7 + [4]
NROWS = [128, 128, 44]

# dtype knobs for the big matmuls: F32 (exact, 4 cy/row) or F32R (1 cy/row)
CFG = {
    "simq": F32,
    "nbr": F32,
    "logits": F32,
    "mm_small": F32,   # sims/colsums stay fp32
}


def _mmcast(ap, dt):
    return ap.bitcast(dt) if dt != F32 else ap


# ---------------------------------------------------------------------------
# The nix walrus in this container only accepts a small number of sync-wait
# commands per instruction; newer Tile emits up to ~27 on the tail drain and
# 3-5 on some body instructions.  Split excess waits onto preceding same-
# engine NoOps.
# ---------------------------------------------------------------------------
WAIT_LIMIT = 1
_wsplit_n = [0]
_PATCHED = [False]


def _patch_tile_wait_limits():
    if _PATCHED[0]:
        return
    _PATCHED[0] = True
    import bass_rust
    from concourse.vector_clock import ScopedClock

    orig_add = tile.TileContext._add_instruction

    def _make_nop(nc_obj, engine, waits):
        nop = bass_rust.InstNoOp(name=f"I-wsplit{_wsplit_n[0]}", ins=[], outs=[])
        _wsplit_n[0] += 1
        nop.engine = engine
        nop.sync_info = mybir.SyncInfo(on_wait=list(waits), on_update=[])
        return nop

    def patched_add(self, inst):
        si = inst.sync_info
        if si is not None and si.on_wait is not None and len(si.on_wait) > WAIT_LIMIT:
            waits = list(si.on_wait)
            head, keep = waits[:-WAIT_LIMIT], waits[-WAIT_LIMIT:]
            for j in range(0, len(head), WAIT_LIMIT):
                orig_add(self, _make_nop(self.nc, inst.engine, head[j:j + WAIT_LIMIT]))
            si.on_wait = keep
        orig_add(self, inst)

    tile.TileContext._add_instruction = patched_add

    def patched_drain(self, tick_clock, wait_clock):
        probe = self.nc.sync.nop()
        wait_clock.add_sem_waits(
            probe.ins, ScopedClock({None: tick_clock.global_clock})
        )
        psi = probe.ins.sync_info
        waits = list(psi.on_wait) if (psi is not None and psi.on_wait) else []
        chunks = [waits[i:i + WAIT_LIMIT] for i in range(0, len(waits), WAIT_LIMIT)]
        if chunks:
            psi.on_wait = chunks[0]
            for ch in chunks[1:]:
                extra = self.nc.sync.nop()
                extra.ins.sync_info = mybir.SyncInfo(on_wait=list(ch), on_update=[])
        self.nc.sync.drain()
        self.nc.all_engine_barrier()
        assert self.sems is not None
        popped = self.nc._tile_sem_poison_stack.pop()
        assert popped is self._sem_poison
        self.nc.clear_and_free_semaphores(list(self.sems.allocated().values()))
        self.nc.all_engine_barrier()

    tile.TileContext._drain_and_barrier = patched_drain


_patch_tile_wait_limits()


def build_nc(cfg=CFG):
    nc = bass.Bass(
        "TRN2",
        target_bir_lowering=False,
        debug=False,
        enable_asserts=False,
        num_devices=NCORES,
    )
    obj_d = nc.dram_tensor("obj", [BL, Q, D], F32, kind="ExternalInput")
    idx_d = nc.dram_tensor("midx", [BL, Nm], I32, kind="ExternalInput")  # pre-adjusted +b*900
    lab_d = nc.dram_tensor("mlab", [BL, Nm], I32, kind="ExternalInput")
    pro_d = nc.dram_tensor("protos", [C, D], F32, kind="ExternalInput")
    w_d = nc.dram_tensor("wcls", [NC, D], F32, kind="ExternalInput")
    b_d = nc.dram_tensor("bcls", [1, NC], F32, kind="ExternalInput")
    id_d = nc.dram_tensor("identc", [128, 128], F32, kind="ExternalInput")
    io90_d = nc.dram_tensor("iota90c", [128, C], F32, kind="ExternalInput")
    qio_d = nc.dram_tensor("qiotac", [128, NQT], F32, kind="ExternalInput")
    out_d = nc.dram_tensor("out", [2], F32, kind="ExternalOutput")

    ar1_in = nc.dram_tensor("ar1_in", [1, 96], F32)
    ar1_out = nc.dram_tensor("ar1_out", [1, 96], F32, addr_space="Shared")
    ar2_in = nc.dram_tensor("ar2_in", [1, 8], F32)
    ar2_out = nc.dram_tensor("ar2_out", [1, 8], F32, addr_space="Shared")
    groups = [list(range(NCORES))]

    obj_flat = obj_d.ap().rearrange("b q d -> (b q) d")

    with tile.TileContext(nc) as tc:
        with (
            tc.tile_pool(name="const", bufs=1) as cp,
            tc.tile_pool(name="big", bufs=2) as bigp,        # obj / objn [128, 2048]
            tc.tile_pool(name="objnT", bufs=2) as otp,       # [128, 1800]
            tc.tile_pool(name="sq", bufs=2) as sqp,          # simq [128,900] x3
            tc.tile_pool(name="mh", bufs=1) as mhp,          # multihot [128,900] x3
            tc.tile_pool(name="mhT", bufs=1) as mhtp,        # [128, 2400]
            tc.tile_pool(name="med", bufs=2) as medp,        # matched & friends
            tc.tile_pool(name="med1", bufs=1) as medp1,      # bcasts, rawT
            tc.tile_pool(name="small", bufs=2) as smp,       # columns / rows
            tc.tile_pool(name="junk", bufs=2) as jkp,        # scratch outputs
            tc.tile_pool(name="acc", bufs=1) as accp,        # persistent accumulators
            tc.tile_pool(name="ps_sq", bufs=2, space="PSUM") as ps_sq,    # [128,900] = 2 banks
            tc.tile_pool(name="ps_nbr", bufs=1, space="PSUM") as ps_nbr,  # [128,300]
            tc.tile_pool(name="ps_sm", bufs=3, space="PSUM") as ps_sm,    # [128,<=300]
        ):
            # alternate psum->sbuf copies between DVE and ACT
            cp_state = [0]

            def col_bcast(dst, col, r, id_sb):
                """dst[128, :r] = col[:r] broadcast across partitions (PE transpose)."""
                pt = ps_sm.tile([128, 300], F32, tag="pst")
                nc.tensor.transpose(
                    out=pt[:, :r], in_=col[:r, :1].to_broadcast([r, 128]),
                    identity=id_sb[:r, :r],
                )
                copy_out(dst, pt[:, :r])

            def copy_out(dst, src):
                cp_state[0] ^= 1
                if cp_state[0]:
                    nc.vector.tensor_copy(dst, src)
                else:
                    nc.scalar.activation(dst, src, AF.Copy)

            # ---------------- constants ----------------
            id_sb = cp.tile([128, 128], F32)
            nc.sync.dma_start(out=id_sb[:, :], in_=id_d.ap()[:, :])
            id_sb_r = cp.tile([128, 128], F32R)
            nc.vector.tensor_copy(id_sb_r[:, :], id_sb[:, :])
            io90 = cp.tile([128, C], F32)
            nc.sync.dma_start(out=io90[:, :], in_=io90_d.ap()[:, :])
            qio = cp.tile([128, NQT], F32)
            nc.sync.dma_start(out=qio[:, :], in_=qio_d.ap()[:, :])
            ones_col = cp.tile([128, 1], F32)
            nc.vector.memset(ones_col[:, :], 1.0)
            ones_row = cp.tile([1, 128], F32)
            nc.vector.memset(ones_row[:, :], 1.0)
            bcls_sb = cp.tile([1, NC], F32)
            nc.sync.dma_start(out=bcls_sb[:, :], in_=b_d.ap()[:, :])
            nshift_col = cp.tile([128, 1], F32)
            nc.vector.memset(nshift_col[:, :], -SHIFT)

            # prototypes [90, 256] -> proT [128, 180] (two d-halves)
            pro_sb = cp.tile([C, D], F32)
            nc.sync.dma_start(out=pro_sb[:, :], in_=pro_d.ap()[:, :])
            proT = cp.tile([128, 2 * C], F32)
            for h in range(2):
                pt = ps_sm.tile([128, C], F32, tag="pst")
                nc.tensor.transpose(
                    out=pt[:, :], in_=pro_sb[:, h * 128:(h + 1) * 128],
                    identity=id_sb[:C, :C],
                )
                copy_out(proT[:, h * C:(h + 1) * C], pt[:, :])

            # W_cls [91, 256] -> wT [128, 182]
            w_sb = cp.tile([NC, D], F32)
            nc.sync.dma_start(out=w_sb[:, :], in_=w_d.ap()[:, :])
            wT = cp.tile([128, 2 * NC], F32)
            for h in range(2):
                pt = ps_sm.tile([128, NC], F32, tag="pst")
                nc.tensor.transpose(
                    out=pt[:, :], in_=w_sb[:, h * 128:(h + 1) * 128],
                    identity=id_sb[:NC, :NC],
                )
                copy_out(wT[:, h * NC:(h + 1) * NC], pt[:, :])

            # P = protos @ protos.T / TAU, diag masked; lse over rows (symmetric)
            pP = ps_sm.tile([C, C], F32, tag="pst")
            for h in range(2):
                nc.tensor.matmul(
                    out=pP[:, :], lhsT=proT[:, h * C:(h + 1) * C],
                    rhs=proT[:, h * C:(h + 1) * C], start=(h == 0), stop=(h == 1),
                )
            P_sb = cp.tile([C, C], F32)
            # P/TAU - 1e9*I
            idbig = cp.tile([C, C], F32)
            nc.vector.tensor_scalar(
                out=idbig[:, :], in0=id_sb[:C, :C], scalar1=1e9, scalar2=None,
                op0=OP.mult,
            )
            nc.vector.tensor_scalar(
                out=P_sb[:, :], in0=pP[:, :], scalar1=1.0 / TAU, scalar2=None,
                op0=OP.mult,
            )
            nc.vector.tensor_tensor(out=P_sb[:, :], in0=P_sb[:, :], in1=idbig[:, :], op=OP.subtract)
            pmax = cp.tile([C, 1], F32)
            nc.vector.tensor_reduce(out=pmax[:, :], in_=P_sb[:, :], axis=mybir.AxisListType.X, op=OP.max)
            npmax = cp.tile([C, 1], F32)
            nc.vector.tensor_scalar(out=npmax[:, :], in0=pmax[:, :], scalar1=-1.0, scalar2=None, op0=OP.mult)
            pexp = cp.tile([C, C], F32)
            psum_col = cp.tile([C, 1], F32)
            nc.scalar.activation(pexp[:, :], P_sb[:, :], AF.Exp, bias=npmax[:, :1], scale=1.0, accum_out=psum_col[:, :1])
            plog = cp.tile([C, 1], F32)
            nc.scalar.activation(plog[:, :], psum_col[:, :], AF.Ln)
            lsePm_col = cp.tile([C, 1], F32)
            nc.vector.tensor_tensor(out=lsePm_col[:, :], in0=plog[:, :], in1=pmax[:, :], op=OP.add)


            # persistent accumulators: cols 0:90 expnet | 90 sul_num | 91 sul_cnt | 92 cec
            acc = accp.tile([128, 96], F32)
            nc.vector.memset(acc[:, :], 0.0)
            labf_all = accp.tile([128, BL * NMT], F32)
            posc_all = accp.tile([128, BL * NMT], F32)

            # ---------------- phase 1: per image ----------------
            for b in range(BL):
                # ---- loads ----
                obj_sb = bigp.tile([128, NQT * D], F32, tag="obj")
                nc.gpsimd.memset(obj_sb[:, 7 * D:], 0.0)
                nc.sync.dma_start(
                    out=obj_sb[:, :7 * D].rearrange("p (t d) -> p t d", d=D),
                    in_=obj_d.ap()[b, :7 * 128, :].rearrange("(t p) d -> p t d", p=128),
                )
                nc.sync.dma_start(out=obj_sb[:4, 7 * D:], in_=obj_d.ap()[b, 7 * 128:, :])

                idxc = smp.tile([128, NMT], I32, tag="idxc")
                nc.gpsimd.memset(idxc[:, :], 0)
                labc = smp.tile([128, NMT], I32, tag="labc")
                nc.gpsimd.memset(labc[:, :], 1 << 30)
                for m in range(NMT):
                    r = NROWS[m]
                    nc.sync.dma_start(
                        out=idxc[:r, m:m + 1],
                        in_=idx_d.ap()[b, m * 128:m * 128 + r].rearrange("(p o) -> p o", o=1),
                    )
                    nc.sync.dma_start(
                        out=labc[:r, m:m + 1],
                        in_=lab_d.ap()[b, m * 128:m * 128 + r].rearrange("(p o) -> p o", o=1),
                    )
                idxcf = smp.tile([128, NMT], F32, tag="idxcf")
                nc.vector.tensor_copy(idxcf[:, :], idxc[:, :])
                for m in range(NMT):
                    nc.vector.tensor_copy(labf_all[:, b * NMT + m: b * NMT + m + 1], labc[:, m:m + 1])

                # ---- matched gather (indices pre-adjusted by +b*900 host-side) ----
                matched = medp.tile([128, NMT * D], F32, tag="matched")
                for m in range(NMT):
                    r = NROWS[m]
                    nc.gpsimd.indirect_dma_start(
                        out=matched[:r, m * D:(m + 1) * D],
                        out_offset=None,
                        in_=obj_flat[:, :],
                        in_offset=bass.IndirectOffsetOnAxis(ap=idxc[:r, m:m + 1], axis=0),
                    )

                # ---- norms ----
                q2 = smp.tile([128, NQT], F32, tag="q2")
                for t in range(NQT):
                    jt = jkp.tile([128, D], F32, tag="j256")
                    nc.scalar.activation(
                        jt[:, :], obj_sb[:, t * D:(t + 1) * D], AF.Square,
                        accum_out=q2[:, t:t + 1],
                    )
                qn = smp.tile([128, NQT], F32, tag="qn")
                nc.scalar.activation(qn[:, :], q2[:, :], AF.Sqrt)
                nc.vector.tensor_scalar(out=qn[:, :], in0=qn[:, :], scalar1=1e-12, scalar2=None, op0=OP.max)
                rq = smp.tile([128, NQT], F32, tag="rq")
                nc.vector.reciprocal(rq[:, :], qn[:, :])

                # is_matched via compare with broadcast idx row
                idx_bc = medp1.tile([128, Nm], F32, tag="idxbc")
                for m in range(NMT):
                    col_bcast(idx_bc[:, m * 128: m * 128 + NROWS[m]], idxcf[:, m:m + 1], NROWS[m], id_sb)
                qio_b = smp.tile([128, NQT], F32, tag="qiob")
                nc.vector.tensor_scalar(out=qio_b[:, :], in0=qio[:, :], scalar1=float(b * Q), scalar2=None, op0=OP.add)
                ism = smp.tile([128, NQT], F32, tag="ism")
                for t in range(NQT):
                    jt = jkp.tile([128, Nm], F32, tag="j300")
                    nc.vector.tensor_scalar(
                        out=jt[:, :], in0=idx_bc[:, :], scalar1=qio_b[:, t:t + 1],
                        scalar2=None, op0=OP.is_equal, op1=OP.add,
                        accum_out=ism[:, t:t + 1],
                    )
                # rqm = rq * (1 - ism)
                rqm = smp.tile([128, NQT], F32, tag="rqm")
                nc.vector.tensor_scalar(out=rqm[:, :], in0=ism[:, :], scalar1=-1.0, scalar2=1.0, op0=OP.mult, op1=OP.add)
                nc.vector.tensor_tensor(out=rqm[:, :], in0=rqm[:, :], in1=rq[:, :], op=OP.mult)

                objn = bigp.tile([128, NQT * D], F32R, tag="objn")
                for t in range(NQT):
                    nc.scalar.activation(
                        objn[:, t * D:(t + 1) * D], obj_sb[:, t * D:(t + 1) * D],
                        AF.Copy, scale=rqm[:, t:t + 1],
                    )

                # matched norms + normalize
                m2 = smp.tile([128, NMT], F32, tag="m2")
                nc.vector.memset(m2[:, :], 1.0)
                for m in range(NMT):
                    r = NROWS[m]
                    jt = jkp.tile([128, D], F32, tag="j256")
                    nc.scalar.activation(
                        jt[:r, :], matched[:r, m * D:(m + 1) * D], AF.Square,
                        accum_out=m2[:r, m:m + 1],
                    )
                mn = smp.tile([128, NMT], F32, tag="mn")
                nc.scalar.activation(mn[:, :], m2[:, :], AF.Sqrt)
                nc.vector.tensor_scalar(out=mn[:, :], in0=mn[:, :], scalar1=1e-12, scalar2=None, op0=OP.max)
                rm = smp.tile([128, NMT], F32, tag="rm")
                nc.vector.reciprocal(rm[:, :], mn[:, :])
                matched_n = medp.tile([128, NMT * D], F32, tag="matchedn")
                for m in range(NMT):
                    r = NROWS[m]
                    nc.scalar.activation(
                        matched_n[:r, m * D:(m + 1) * D], matched[:r, m * D:(m + 1) * D],
                        AF.Copy, scale=rm[:r, m:m + 1],
                    )

                # ---- transposes: matched_nT, matchedT [128, 600], objnT [128, 1800] ----
                mnT = medp.tile([128, 2 * Nm], F32, tag="mnT")
                mnT_r = medp.tile([128, 2 * Nm], F32R, tag="mnTr")
                mT = medp1.tile([128, 2 * Nm], F32, tag="mT")
                for m in range(NMT):
                    r = NROWS[m]
                    for h in range(2):
                        pt = ps_sm.tile([128, 300], F32, tag="pst")
                        nc.tensor.transpose(
                            out=pt[:, :r],
                            in_=matched_n[:r, m * D + h * 128: m * D + (h + 1) * 128],
                            identity=id_sb[:r, :r],
                        )
                        copy_out(mnT[:, h * Nm + m * 128: h * Nm + m * 128 + r], pt[:, :r])
                        copy_out(mnT_r[:, h * Nm + m * 128: h * Nm + m * 128 + r], pt[:, :r])
                        pt2 = ps_sm.tile([128, 300], F32, tag="pst")
                        nc.tensor.transpose(
                            out=pt2[:, :r],
                            in_=matched[:r, m * D + h * 128: m * D + (h + 1) * 128],
                            identity=id_sb[:r, :r],
                        )
                        copy_out(mT[:, h * Nm + m * 128: h * Nm + m * 128 + r], pt2[:, :r])

                objnT = otp.tile([128, 2 * Q], F32R, tag="objnT")
                for t in range(NQT):
                    r = QROWS[t]
                    for h in range(2):
                        pt = ps_sm.tile([128, 300], F32R, tag="pst")
                        nc.tensor.transpose(
                            out=pt[:, :r],
                            in_=objn[:r, t * D + h * 128: t * D + (h + 1) * 128],
                            identity=id_sb_r[:r, :r],
                        )
                        copy_out(objnT[:, h * Q + t * 128: h * Q + t * 128 + r], pt[:, :r])

                # ---- sims = matched_n @ protos.T  [300, 90] ----
                sims_sb = medp.tile([128, NMT * C], F32, tag="sims")
                for m in range(NMT):
                    r = NROWS[m]
                    psim = ps_sm.tile([128, 300], F32, tag="pst")
                    for h in range(2):
                        nc.tensor.matmul(
                            out=psim[:r, :C],
                            lhsT=mnT[:, h * Nm + m * 128: h * Nm + m * 128 + r],
                            rhs=proT[:, h * C:(h + 1) * C],
                            start=(h == 0), stop=(h == 1),
                        )
                    copy_out(sims_sb[:r, m * C:(m + 1) * C], psim[:r, :C])

                # ---- simQ = matched_n @ obj_n.T  [300, 900] ----
                simq_sb = sqp.tile([128, NMT * Q], F32, tag="simq")
                for m in range(NMT):
                    r = NROWS[m]
                    psq = ps_sq.tile([128, Q], F32, tag="psq")
                    for c0, c1 in ((0, 512), (512, Q)):
                        for h in range(2):
                            nc.tensor.matmul(
                                out=psq[:r, c0:c1],
                                lhsT=mnT_r[:, h * Nm + m * 128: h * Nm + m * 128 + r],
                                rhs=objnT[:, h * Q + c0: h * Q + c1],
                                start=(h == 0), stop=(h == 1),
                            )
                    copy_out(simq_sb[:r, m * Q:(m + 1) * Q], psq[:r, :])

                # ---- top-5 threshold, multihot, wcnt ----
                mh = mhp.tile([128, NMT * Q], F32R, tag="mh")
                wcnt = smp.tile([128, NMT], F32, tag="wcnt")
                nc.vector.memset(wcnt[:, :], 0.0)
                thr = smp.tile([128, NMT], F32, tag="thr")
                for m in range(NMT):
                    r = NROWS[m]
                    mx8 = jkp.tile([128, 8], F32, tag="mx8")
                    nc.vector.max(out=mx8[:r, :], in_=simq_sb[:r, m * Q:(m + 1) * Q])
                    nc.vector.tensor_scalar(out=thr[:r, m:m + 1], in0=mx8[:r, 4:5], scalar1=1e-30, scalar2=None, op0=OP.max)
                    nc.vector.tensor_scalar(
                        out=mh[:r, m * Q:(m + 1) * Q], in0=simq_sb[:r, m * Q:(m + 1) * Q],
                        scalar1=thr[:r, m:m + 1], scalar2=None,
                        op0=OP.is_ge, op1=OP.add, accum_out=wcnt[:r, m:m + 1],
                    )

                # ---- multihot transpose [q, n] ----
                mhT = mhtp.tile([128, NQT * Nm], F32R, tag="mhT")
                for m in range(NMT):
                    r = NROWS[m]
                    for t in range(NQT):
                        qr = QROWS[t]
                        pt = ps_sm.tile([128, 300], F32R, tag="pst")
                        nc.tensor.transpose(
                            out=pt[:qr, :r],
                            in_=mh[:r, m * Q + t * 128: m * Q + t * 128 + qr],
                            identity=id_sb_r[:r, :r],
                        )
                        # scale rows by ||obj_q|| so that objn @ mhT_w == obj @ multihot.T
                        nc.vector.tensor_scalar(
                            out=mhT[:qr, t * Nm + m * 128: t * Nm + m * 128 + r],
                            in0=pt[:qr, :r], scalar1=qn[:qr, t:t + 1], scalar2=None,
                            op0=OP.mult,
                        )

                # ---- nbr sum: rawT = matchedT + obj.T @ multihot.T  [256 x 300] ----
                rawT = medp1.tile([128, 2 * Nm], F32, tag="rawT")
                for h in range(2):
                    pn = ps_nbr.tile([128, Nm], F32, tag="pnbr")
                    for t in range(NQT):
                        qr = QROWS[t]
                        nc.tensor.matmul(
                            out=pn[:, :],
                            lhsT=objn[:qr, t * D + h * 128: t * D + (h + 1) * 128],
                            rhs=mhT[:qr, t * Nm:(t + 1) * Nm],
                            start=(t == 0), stop=(t == NQT - 1),
                        )
                    nc.vector.tensor_tensor(
                        out=rawT[:, h * Nm:(h + 1) * Nm], in0=pn[:, :],
                        in1=mT[:, h * Nm:(h + 1) * Nm], op=OP.add,
                    )

                # ---- logits & focal ----
                den = smp.tile([128, NMT], F32, tag="den")
                nc.vector.tensor_scalar(out=den[:, :], in0=wcnt[:, :], scalar1=1.0, scalar2=None, op0=OP.add)
                sden = smp.tile([128, NMT], F32, tag="sden")
                nc.vector.reciprocal(sden[:, :], den[:, :])

                fl = smp.tile([128, NMT], F32, tag="fl")
                hasn = smp.tile([128, NMT], F32, tag="hasn")
                nc.vector.tensor_scalar(out=hasn[:, :], in0=wcnt[:, :], scalar1=0.5, scalar2=None, op0=OP.is_gt)

                for m in range(NMT):
                    r = NROWS[m]
                    pl = ps_sm.tile([128, 300], F32, tag="pst")
                    for h in range(2):
                        nc.tensor.matmul(
                            out=pl[:r, :NC],
                            lhsT=rawT[:, h * Nm + m * 128: h * Nm + m * 128 + r],
                            rhs=wT[:, h * NC:(h + 1) * NC],
                            start=(h == 0), stop=False,
                        )
                    nc.tensor.matmul(
                        out=pl[:r, :NC], lhsT=ones_row[:1, :r], rhs=bcls_sb[:1, :],
                        start=False, stop=True,
                    )
                    lg = jkp.tile([128, NC], F32, tag="lg")
                    nc.vector.tensor_scalar(out=lg[:r, :], in0=pl[:r, :NC], scalar1=sden[:r, m:m + 1], scalar2=None, op0=OP.mult)
                    # focal: X_j = softplus(l)*sig(l)^2 ; last col uses -l
                    sg = jkp.tile([128, NC], F32, tag="sg")
                    nc.scalar.activation(sg[:r, :], lg[:r, :], AF.Sigmoid)
                    sp = jkp.tile([128, NC], F32, tag="sp")
                    nc.scalar.activation(sp[:r, :], lg[:r, :], AF.Exp)
                    nc.scalar.activation(sp[:r, :], sp[:r, :], AF.Ln, bias=1.0, scale=1.0)
                    s2 = jkp.tile([128, NC], F32, tag="s2")
                    nc.vector.tensor_tensor(out=s2[:r, :], in0=sg[:r, :], in1=sg[:r, :], op=OP.mult)
                    X = jkp.tile([128, NC], F32, tag="X")
                    xs = jkp.tile([128, 1], F32, tag="xs")
                    nc.vector.tensor_tensor(out=X[:r, :], in0=s2[:r, :], in1=sp[:r, :], op=OP.mult)
                    nc.vector.tensor_reduce(out=xs[:r, :1], in_=X[:r, :], axis=mybir.AxisListType.X, op=OP.add)
                    # Y = softplus(-l_last)*sig(-l_last)^2
                    nl = jkp.tile([128, 1], F32, tag="nl")
                    nc.vector.tensor_scalar(out=nl[:r, :], in0=lg[:r, NC - 1:NC], scalar1=-1.0, scalar2=None, op0=OP.mult)
                    sgn = jkp.tile([128, 1], F32, tag="sgn")
                    nc.scalar.activation(sgn[:r, :], nl[:r, :], AF.Sigmoid)
                    spn = jkp.tile([128, 1], F32, tag="spn")
                    nc.scalar.activation(spn[:r, :], nl[:r, :], AF.Exp)
                    nc.scalar.activation(spn[:r, :], spn[:r, :], AF.Ln, bias=1.0, scale=1.0)
                    Y = jkp.tile([128, 1], F32, tag="Y")
                    nc.vector.tensor_tensor(out=Y[:r, :], in0=sgn[:r, :], in1=sgn[:r, :], op=OP.mult)
                    nc.vector.tensor_tensor(out=Y[:r, :], in0=Y[:r, :], in1=spn[:r, :], op=OP.mult)
                    # fl = (0.75*(xs - X_last) + 0.25*Y)/NC
                    t1 = jkp.tile([128, 1], F32, tag="t1")
                    nc.vector.tensor_tensor(out=t1[:r, :], in0=xs[:r, :], in1=X[:r, NC - 1:NC], op=OP.subtract)
                    nc.vector.tensor_scalar(out=t1[:r, :], in0=t1[:r, :], scalar1=0.75 / NC, scalar2=None, op0=OP.mult)
                    nc.vector.tensor_scalar(out=Y[:r, :], in0=Y[:r, :], scalar1=0.25 / NC, scalar2=None, op0=OP.mult)
                    nc.vector.tensor_tensor(out=fl[:r, m:m + 1], in0=t1[:r, :], in1=Y[:r, :], op=OP.add)

                # ---- pos, dists, CEC exp accumulation, rank-in-class ----
                dcol = smp.tile([128, NMT], F32, tag="dcol")
                for m in range(NMT):
                    r = NROWS[m]
                    mask = jkp.tile([128, C], F32, tag="mask")
                    nc.vector.tensor_scalar(
                        out=mask[:r, :], in0=io90[:r, :], scalar1=labf_all[:r, b * NMT + m: b * NMT + m + 1],
                        scalar2=None, op0=OP.is_equal,
                    )
                    j90 = jkp.tile([128, C], F32, tag="j90")
                    nc.vector.tensor_tensor(out=j90[:r, :], in0=sims_sb[:r, m * C:(m + 1) * C], in1=mask[:r, :], op=OP.mult)
                    nc.vector.tensor_reduce(out=posc_all[:r, b * NMT + m: b * NMT + m + 1], in_=j90[:r, :], axis=mybir.AxisListType.X, op=OP.add)
                    nc.vector.tensor_scalar(
                        out=dcol[:r, m:m + 1], in0=posc_all[:r, b * NMT + m: b * NMT + m + 1],
                        scalar1=-1.0, scalar2=1.0, op0=OP.mult, op1=OP.add,
                    )
                    # expnet += exp(10*sims - 10) * (1 - mask)
                    expm = jkp.tile([128, C], F32, tag="expm")
                    nc.scalar.activation(expm[:r, :], sims_sb[:r, m * C:(m + 1) * C], AF.Exp, bias=nshift_col[:r, :1], scale=1.0 / TAU)
                    nm_ = jkp.tile([128, C], F32, tag="nm_")
                    nc.vector.tensor_scalar(out=nm_[:r, :], in0=mask[:r, :], scalar1=-1.0, scalar2=1.0, op0=OP.mult, op1=OP.add)
                    nc.vector.tensor_tensor(out=expm[:r, :], in0=expm[:r, :], in1=nm_[:r, :], op=OP.mult)
                    nc.vector.tensor_tensor(out=acc[:r, 0:C], in0=acc[:r, 0:C], in1=expm[:r, :], op=OP.add)

                d_bc = medp1.tile([128, Nm], F32, tag="dbc")
                lab_bc = medp1.tile([128, Nm], F32, tag="labbc")
                for m in range(NMT):
                    r = NROWS[m]
                    col_bcast(d_bc[:, m * 128: m * 128 + r], dcol[:, m:m + 1], r, id_sb)
                    col_bcast(lab_bc[:, m * 128: m * 128 + r],
                              labf_all[:, b * NMT + m: b * NMT + m + 1], r, id_sb)

                for m in range(NMT):
                    r = NROWS[m]
                    eq = jkp.tile([128, Nm], F32, tag="eq")
                    nc.vector.tensor_scalar(
                        out=eq[:r, :], in0=lab_bc[:r, :],
                        scalar1=labf_all[:r, b * NMT + m: b * NMT + m + 1], scalar2=None, op0=OP.is_equal,
                    )
                    gt = jkp.tile([128, Nm], F32, tag="gt")
                    nc.vector.tensor_scalar(
                        out=gt[:r, :], in0=d_bc[:r, :], scalar1=dcol[:r, m:m + 1],
                        scalar2=None, op0=OP.is_gt,
                    )
                    j300 = jkp.tile([128, Nm], F32, tag="j300b")
                    cnt = jkp.tile([128, 1], F32, tag="cnt")
                    nc.vector.tensor_tensor(out=j300[:r, :], in0=eq[:r, :], in1=gt[:r, :], op=OP.mult)
                    nc.vector.tensor_reduce(out=cnt[:r, :1], in_=j300[:r, :], axis=mybir.AxisListType.X, op=OP.add)
                    sel = jkp.tile([128, 1], F32, tag="sel")
                    nc.vector.tensor_scalar(out=sel[:r, :], in0=cnt[:r, :], scalar1=4.5, scalar2=None, op0=OP.is_lt)
                    c1 = jkp.tile([128, 1], F32, tag="c1")
                    nc.vector.tensor_tensor(out=c1[:r, :], in0=sel[:r, :], in1=hasn[:r, m:m + 1], op=OP.mult)
                    c2 = jkp.tile([128, 1], F32, tag="c2")
                    nc.vector.tensor_tensor(out=c2[:r, :], in0=c1[:r, :], in1=fl[:r, m:m + 1], op=OP.mult)
                    nc.vector.tensor_tensor(out=acc[:r, C:C + 1], in0=acc[:r, C:C + 1], in1=c2[:r, :], op=OP.add)
                    nc.vector.tensor_tensor(out=acc[:r, C + 1:C + 2], in0=acc[:r, C + 1:C + 2], in1=c1[:r, :], op=OP.add)

            # ---------------- AllReduce 1: [expnet(90) | sul_num | sul_cnt] ----------------
            pr1 = ps_sm.tile([1, 300], F32, tag="pst")
            nc.tensor.matmul(out=pr1[:1, :92], lhsT=ones_col[:, :1], rhs=acc[:, 0:92], start=True, stop=True)
            r1 = smp.tile([1, 96], F32, tag="r1")
            nc.vector.memset(r1[:, :], 0.0)
            nc.vector.tensor_copy(r1[:1, :92], pr1[:1, :92])
            nc.sync.dma_start(out=ar1_in.ap()[:, :], in_=r1[:, :])
            nc.gpsimd.collective_compute(
                "AllReduce", OP.add, replica_groups=groups,
                ins=[ar1_in.ap()[:, :]], outs=[ar1_out.ap()[:, :]],
            )
            g1 = smp.tile([96, 1], F32, tag="g1")
            nc.sync.dma_start(out=g1[:, :], in_=ar1_out.ap()[0, :].rearrange("(p o) -> p o", o=1))

            # lse_neg col = logaddexp(lsePm, SHIFT + ln(sumexp))
            lnS = smp.tile([C, 1], F32, tag="lnS")
            nc.scalar.activation(lnS[:, :], g1[:C, :], AF.Ln)
            nc.vector.tensor_scalar(out=lnS[:, :], in0=lnS[:, :], scalar1=SHIFT, scalar2=None, op0=OP.add)
            mx = smp.tile([C, 1], F32, tag="mx")
            nc.vector.tensor_tensor(out=mx[:, :], in0=lnS[:, :], in1=lsePm_col[:, :], op=OP.max)
            mnm = smp.tile([C, 1], F32, tag="mnm")
            nc.vector.tensor_tensor(out=mnm[:, :], in0=lnS[:, :], in1=lsePm_col[:, :], op=OP.min)
            nc.vector.tensor_tensor(out=mnm[:, :], in0=mnm[:, :], in1=mx[:, :], op=OP.subtract)
            ef = smp.tile([C, 1], F32, tag="ef")
            nc.scalar.activation(ef[:, :], mnm[:, :], AF.Exp)
            l1 = smp.tile([C, 1], F32, tag="l1")
            nc.scalar.activation(l1[:, :], ef[:, :], AF.Ln, bias=1.0, scale=1.0)
            lneg = smp.tile([C, 1], F32, tag="lneg")
            nc.vector.tensor_tensor(out=lneg[:, :], in0=mx[:, :], in1=l1[:, :], op=OP.add)
            ln_bc = medp1.tile([128, C], F32, tag="lnbc")
            col_bcast(ln_bc[:, :C], lneg[:, :1], C, id_sb)

            # ---------------- phase 2: per-sample CEC ----------------
            for b in range(BL):
                for m in range(NMT):
                    r = NROWS[m]
                    k = b * NMT + m
                    mask = jkp.tile([128, C], F32, tag="mask")
                    nc.vector.tensor_scalar(
                        out=mask[:r, :], in0=io90[:r, :], scalar1=labf_all[:r, k:k + 1],
                        scalar2=None, op0=OP.is_equal,
                    )
                    j90 = jkp.tile([128, C], F32, tag="j90")
                    lnn = jkp.tile([128, 1], F32, tag="lnn")
                    nc.vector.tensor_tensor(out=j90[:r, :], in0=ln_bc[:r, :], in1=mask[:r, :], op=OP.mult)
                    nc.vector.tensor_reduce(out=lnn[:r, :1], in_=j90[:r, :], axis=mybir.AxisListType.X, op=OP.add)
                    posS = jkp.tile([128, 1], F32, tag="posS")
                    nc.vector.tensor_scalar(out=posS[:r, :], in0=posc_all[:r, k:k + 1], scalar1=1.0 / TAU, scalar2=None, op0=OP.mult)
                    mxc = jkp.tile([128, 1], F32, tag="mxc")
                    nc.vector.tensor_tensor(out=mxc[:r, :], in0=posS[:r, :], in1=lnn[:r, :], op=OP.max)
                    mnc = jkp.tile([128, 1], F32, tag="mnc")
                    nc.vector.tensor_tensor(out=mnc[:r, :], in0=posS[:r, :], in1=lnn[:r, :], op=OP.min)
                    nc.vector.tensor_tensor(out=mnc[:r, :], in0=mnc[:r, :], in1=mxc[:r, :], op=OP.subtract)
                    efc = jkp.tile([128, 1], F32, tag="efc")
                    nc.scalar.activation(efc[:r, :], mnc[:r, :], AF.Exp)
                    l1c = jkp.tile([128, 1], F32, tag="l1c")
                    nc.scalar.activation(l1c[:r, :], efc[:r, :], AF.Ln, bias=1.0, scale=1.0)
                    nc.vector.tensor_tensor(out=mxc[:r, :], in0=mxc[:r, :], in1=l1c[:r, :], op=OP.add)
                    nc.vector.tensor_tensor(out=mxc[:r, :], in0=mxc[:r, :], in1=posS[:r, :], op=OP.subtract)
                    nc.vector.tensor_tensor(out=acc[:r, 92:93], in0=acc[:r, 92:93], in1=mxc[:r, :], op=OP.add)

            # ---------------- AllReduce 2: cec_sum ----------------
            pr2 = ps_sm.tile([1, 300], F32, tag="pst")
            nc.tensor.matmul(out=pr2[:1, :1], lhsT=ones_col[:, :1], rhs=acc[:, 92:93], start=True, stop=True)
            r2 = smp.tile([1, 8], F32, tag="r2")
            nc.vector.memset(r2[:, :], 0.0)
            nc.vector.tensor_copy(r2[:1, :1], pr2[:1, :1])
            nc.sync.dma_start(out=ar2_in.ap()[:, :], in_=r2[:, :])
            nc.gpsimd.collective_compute(
                "AllReduce", OP.add, replica_groups=groups,
                ins=[ar2_in.ap()[:, :]], outs=[ar2_out.ap()[:, :]],
            )
            g2 = smp.tile([1, 8], F32, tag="g2")
            nc.sync.dma_start(out=g2[:, :], in_=ar2_out.ap()[:, :])

            # ---------------- final output ----------------
            outr = smp.tile([1, 2], F32, tag="outr")
            sulrow = smp.tile([1, 2], F32, tag="sulrow")
            nc.sync.dma_start(out=sulrow[:, :], in_=ar1_out.ap()[:, 90:92])
            denf = smp.tile([1, 1], F32, tag="denf")
            nc.vector.tensor_scalar(out=denf[:, :], in0=sulrow[:1, 1:2], scalar1=1.0, scalar2=None, op0=OP.max)
            rdf = smp.tile([1, 1], F32, tag="rdf")
            nc.vector.reciprocal(rdf[:, :], denf[:, :])
            nc.vector.tensor_tensor(out=outr[:1, 0:1], in0=sulrow[:1, 0:1], in1=rdf[:1, :], op=OP.mult)
            nc.vector.tensor_scalar(out=outr[:1, 1:2], in0=g2[:1, 0:1], scalar1=1.0 / (B * Nm), scalar2=None, op0=OP.mult)
            nc.sync.dma_start(out=out_d.ap().rearrange("(a b) -> a b", a=1), in_=outr[:, :])

    return nc


def make_in_maps(obj_embs, prototypes, W_cls, b_cls, match_src_idx, match_labels):
    identc = np.eye(128, dtype=np.float32)
    iota90c = np.tile(np.arange(C, dtype=np.float32), (128, 1))
    qiotac = (np.arange(128, dtype=np.float32)[:, None]
              + 128.0 * np.arange(NQT, dtype=np.float32)[None, :]).astype(np.float32)
    adj = (np.arange(BL, dtype=np.int32) * Q)[:, None]
    in_maps = []
    for c in range(NCORES):
        sl = slice(c * BL, (c + 1) * BL)
        in_maps.append({
            "obj": np.ascontiguousarray(obj_embs[sl]).astype(np.float32),
            "midx": (match_src_idx[sl].astype(np.int32) + adj),
            "mlab": np.ascontiguousarray(match_labels[sl]).astype(np.int32),
            "protos": np.ascontiguousarray(prototypes).astype(np.float32),
            "wcls": np.ascontiguousarray(W_cls).astype(np.float32),
            "bcls": np.ascontiguousarray(b_cls).astype(np.float32).reshape(1, NC),
            "identc": identc,
            "iota90c": iota90c,
            "qiotac": qiotac,
        })
    return in_maps


_CACHE = {}


def _install_ntff_shim():
    """Register the axon NTFF profile hook (test-time only; grading never traces)."""
    import types
    try:
        from antenv.axon_hooks import get_axon_ntff_profile_hook  # noqa: F401
        return
    except ImportError:
        pass
    import antenv
    from trn_agent_boot.trn_boot import _ntff_profile_via_ctypes
    mod = types.ModuleType("antenv.axon_hooks")
    _hook = [None]
    mod.set_axon_ntff_profile_hook = lambda h: _hook.__setitem__(0, h)
    mod.get_axon_ntff_profile_hook = lambda: _hook[0]
    sys.modules["antenv.axon_hooks"] = mod
    antenv.axon_hooks = mod
    mod.set_axon_ntff_profile_hook(_ntff_profile_via_ctypes("/opt/axon/libaxon_pjrt.so"))
    orig_upload = bass_utils.upload_artifacts
    def _safe_upload(tmpdir):
        try:
            return orig_upload(tmpdir)
        except Exception as e:
            print("upload_artifacts skipped:", e)
            return tmpdir
    bass_utils.upload_artifacts = _safe_upload


def kernel(obj_embs, prototypes, W_cls, b_cls, match_src_idx, match_labels,
           _trace=False, **extra):
    if _trace:
        _install_ntff_shim()
    if "nc" not in _CACHE:
        _CACHE["nc"] = build_nc()
    nc = _CACHE["nc"]
    in_maps = make_in_maps(obj_embs, prototypes, W_cls, b_cls,
                           match_src_idx, match_labels)
    res = bass_utils.run_bass_kernel_spmd(
        nc, in_maps, core_ids=list(range(NCORES)), trace=_trace,
    )
    _CACHE["last_results"] = res
    return np.asarray(res.results[0]["out"], dtype=np.float32).reshape(2)


if __name__ == "__main__":
    nc = build_nc()
    print("built ok")


# revision 34
# speedup vs baseline: 1.1801x; 1.1012x over previous
"""Trainium2 Bass kernel for nn_ASGSCriterion (SUL focal loss + CEC InfoNCE).

Strategy (data-parallel over batch, 4 images / core on 8 cores):

The reference's [B, C, K_B] boundary-mining structure collapses to per-matched-row
work: matched row n is a valid boundary sample iff its prototype-distance ranks
in the top K_B=5 within its class (strictly-greater count < 5), and the focal
loss of slot (c,k) equals the per-row focal loss of the selected row (single-row
selection commutes with normalization).  Per image:

  obj_n   = normalize(obj)                           [900, 256]
  matched = gather(obj, idx)   (indirect DMA)        [300, 256]
  sims    = matched_n @ protos.T                     [300, 90]   (pos, dists)
  simQ    = matched_n @ obj_n.T  (matched cols of obj_n zeroed)  [300, 900]
  thr     = max(5th-largest(simQ row), tiny)  -> multihot = simQ >= thr
  nbr     = multihot @ obj  (matmul gather-sum)      [256, 300]
  logits  = ((matchedT + nbrT) * 1/(1+wcnt)) @ W.T + b
  fl      = focal loss per row;  sel = rank-in-class < 5;  has = wcnt > 0
  SUL     = sum(sel*has*fl) / max(sum(sel*has), 1)          (global all-reduce)
  CEC     = InfoNCE with fixed logsumexp shift of 10 (S = cos/tau <= 10)

Two tiny AllReduces: [sumexp(90) | sul_num | sul_cnt] mid-kernel, then cec_sum.
"""

import sys

if "/opt/trn_rl_repo" not in sys.path:
    sys.path.insert(0, "/opt/trn_rl_repo")

import numpy as np

import concourse.bass as bass
import concourse.mybir as mybir
import concourse.tile as tile
from concourse import bass_utils
from concourse import library_config

F32 = mybir.dt.float32
F32R = mybir.dt.float32r
I32 = mybir.dt.int32
AF = mybir.ActivationFunctionType
OP = mybir.AluOpType

B, Q, D, Nm, C, NC = 32, 900, 256, 300, 90, 91
NCORES = 8
BL = B // NCORES          # images per core
TAU = 0.1
SHIFT = 10.0              # fixed logsumexp shift; |S| <= 1/TAU = 10
NQT = 8                   # q tiles (900 -> 7*128 + 4)
NMT = 3                   # n tiles (300 -> 2*128 + 44)
QROWS = [128] * 7 + [4]
NROWS = [128, 128, 44]

# dtype knobs for the big matmuls: F32 (exact, 4 cy/row) or F32R (1 cy/row)
CFG = {
    "simq": F32,
    "nbr": F32,
    "logits": F32,
    "mm_small": F32,   # sims/colsums stay fp32
}


def _mmcast(ap, dt):
    return ap.bitcast(dt) if dt != F32 else ap


# ---------------------------------------------------------------------------
# The nix walrus in this container only accepts a small number of sync-wait
# commands per instruction; newer Tile emits up to ~27 on the tail drain and
# 3-5 on some body instructions.  Split excess waits onto preceding same-
# engine NoOps.
# ---------------------------------------------------------------------------
WAIT_LIMIT = 1
_wsplit_n = [0]
_PATCHED = [False]


def _patch_tile_wait_limits():
    if _PATCHED[0]:
        return
    _PATCHED[0] = True
    import bass_rust
    from concourse.vector_clock import ScopedClock

    orig_add = tile.TileContext._add_instruction

    def _make_nop(nc_obj, engine, waits):
        nop = bass_rust.InstNoOp(name=f"I-wsplit{_wsplit_n[0]}", ins=[], outs=[])
        _wsplit_n[0] += 1
        nop.engine = engine
        nop.sync_info = mybir.SyncInfo(on_wait=list(waits), on_update=[])
        return nop

    def patched_add(self, inst):
        si = inst.sync_info
        if si is not None and si.on_wait is not None and len(si.on_wait) > WAIT_LIMIT:
            waits = list(si.on_wait)
            head, keep = waits[:-WAIT_LIMIT], waits[-WAIT_LIMIT:]
            for j in range(0, len(head), WAIT_LIMIT):
                orig_add(self, _make_nop(self.nc, inst.engine, head[j:j + WAIT_LIMIT]))
            si.on_wait = keep
        orig_add(self, inst)

    tile.TileContext._add_instruction = patched_add

    def patched_drain(self, tick_clock, wait_clock):
        probe = self.nc.sync.nop()
        wait_clock.add_sem_waits(
            probe.ins, ScopedClock({None: tick_clock.global_clock})
        )
        psi = probe.ins.sync_info
        waits = list(psi.on_wait) if (psi is not None and psi.on_wait) else []
        chunks = [waits[i:i + WAIT_LIMIT] for i in range(0, len(waits), WAIT_LIMIT)]
        if chunks:
            psi.on_wait = chunks[0]
            for ch in chunks[1:]:
                extra = self.nc.sync.nop()
                extra.ins.sync_info = mybir.SyncInfo(on_wait=list(ch), on_update=[])
        self.nc.sync.drain()
        self.nc.all_engine_barrier()
        assert self.sems is not None
        popped = self.nc._tile_sem_poison_stack.pop()
        assert popped is self._sem_poison
        self.nc.clear_and_free_semaphores(list(self.sems.allocated().values()))
        self.nc.all_engine_barrier()

    tile.TileContext._drain_and_barrier = patched_drain


_patch_tile_wait_limits()


def build_nc(cfg=CFG):
    nc = bass.Bass(
        "TRN2",
        target_bir_lowering=False,
        debug=False,
        enable_asserts=False,
        num_devices=NCORES,
    )
    obj_d = nc.dram_tensor("obj", [BL, Q, D], F32, kind="ExternalInput")
    idx_d = nc.dram_tensor("midx", [BL, Nm], I32, kind="ExternalInput")  # pre-adjusted +b*900
    idxr_d = nc.dram_tensor("midxraw", [BL, Nm], I32, kind="ExternalInput")
    lab_d = nc.dram_tensor("mlab", [BL, Nm], I32, kind="ExternalInput")
    pro_d = nc.dram_tensor("protos", [C, D], F32, kind="ExternalInput")
    w_d = nc.dram_tensor("wcls", [NC, D], F32, kind="ExternalInput")
    b_d = nc.dram_tensor("bcls", [1, NC], F32, kind="ExternalInput")
    id_d = nc.dram_tensor("identc", [128, 128], F32, kind="ExternalInput")
    io90_d = nc.dram_tensor("iota90c", [128, C], F32, kind="ExternalInput")
    qio_d = nc.dram_tensor("qiotac", [128, NQT], F32, kind="ExternalInput")
    out_d = nc.dram_tensor("out", [2], F32, kind="ExternalOutput")

    ar1_in = nc.dram_tensor("ar1_in", [1, 96], F32)
    ar1_out = nc.dram_tensor("ar1_out", [1, 96], F32, addr_space="Shared")
    ar2_in = nc.dram_tensor("ar2_in", [1, 8], F32)
    ismd = [nc.dram_tensor(f"ismd{i}", [NQT * 128, 1], F32) for i in range(BL)]
    ar2_out = nc.dram_tensor("ar2_out", [1, 8], F32, addr_space="Shared")
    groups = [list(range(NCORES))]

    obj_flat = obj_d.ap().rearrange("b q d -> (b q) d")

    with tile.TileContext(nc) as tc:
        with (
            tc.tile_pool(name="const", bufs=1) as cp,
            tc.tile_pool(name="big", bufs=2) as bigp,        # obj / objn [128, 2048]
            tc.tile_pool(name="objnT", bufs=2) as otp,       # [128, 1800]
            tc.tile_pool(name="sq", bufs=2) as sqp,          # simq [128,900] x3
            tc.tile_pool(name="mh", bufs=2) as mhp,          # multihot [128,900] x3
            tc.tile_pool(name="mhT", bufs=2) as mhtp,        # [128, 2400]
            tc.tile_pool(name="med", bufs=2) as medp,        # matched & friends
            tc.tile_pool(name="med1", bufs=1) as medp1,      # bcasts, rawT
            tc.tile_pool(name="small", bufs=2) as smp,       # columns / rows
            tc.tile_pool(name="junk", bufs=3) as jkp,        # scratch outputs
            tc.tile_pool(name="acc", bufs=1) as accp,        # persistent accumulators
            tc.tile_pool(name="ps_sq", bufs=2, space="PSUM") as ps_sq,    # [128,900] = 2 banks
            tc.tile_pool(name="ps_nbr", bufs=1, space="PSUM") as ps_nbr,  # [128,300]
            tc.tile_pool(name="ps_sm", bufs=3, space="PSUM") as ps_sm,    # [128,<=300]
        ):
            # alternate psum->sbuf copies between DVE and ACT
            cp_state = [0]

            def col_bcast(dst, col, r, id_sb):
                """dst[128, :r] = col[:r] broadcast across partitions (PE transpose)."""
                pt = ps_sm.tile([128, 300], F32, tag="pst")
                nc.tensor.transpose(
                    out=pt[:, :r], in_=col[:r, :1].to_broadcast([r, 128]),
                    identity=id_sb[:r, :r],
                )
                copy_out(dst, pt[:, :r])

            def copy_out(dst, src):
                cp_state[0] ^= 1
                if cp_state[0]:
                    nc.vector.tensor_copy(dst, src)
                else:
                    nc.scalar.activation(dst, src, AF.Copy)

            # ---------------- constants ----------------
            id_sb = cp.tile([128, 128], F32)
            nc.sync.dma_start(out=id_sb[:, :], in_=id_d.ap()[:, :])
            id_sb_r = cp.tile([128, 128], F32R)
            nc.vector.tensor_copy(id_sb_r[:, :], id_sb[:, :])
            io90 = cp.tile([128, C], F32)
            nc.sync.dma_start(out=io90[:, :], in_=io90_d.ap()[:, :])
            qio = cp.tile([128, NQT], F32)
            nc.sync.dma_start(out=qio[:, :], in_=qio_d.ap()[:, :])
            ones_col = cp.tile([128, 1], F32)
            nc.vector.memset(ones_col[:, :], 1.0)
            ones_row = cp.tile([1, 128], F32)
            nc.vector.memset(ones_row[:, :], 1.0)
            bcls_sb = cp.tile([1, NC], F32)
            nc.sync.dma_start(out=bcls_sb[:, :], in_=b_d.ap()[:, :])
            nshift_col = cp.tile([128, 1], F32)
            nc.vector.memset(nshift_col[:, :], -SHIFT)

            # prototypes [90, 256] -> proT [128, 180] (two d-halves)
            pro_sb = cp.tile([C, D], F32)
            nc.sync.dma_start(out=pro_sb[:, :], in_=pro_d.ap()[:, :])
            proT = cp.tile([128, 2 * C], F32)
            for h in range(2):
                pt = ps_sm.tile([128, C], F32, tag="pst")
                nc.tensor.transpose(
                    out=pt[:, :], in_=pro_sb[:, h * 128:(h + 1) * 128],
                    identity=id_sb[:C, :C],
                )
                copy_out(proT[:, h * C:(h + 1) * C], pt[:, :])

            # W_cls [91, 256] -> wT [128, 182]
            w_sb = cp.tile([NC, D], F32)
            nc.sync.dma_start(out=w_sb[:, :], in_=w_d.ap()[:, :])
            wT = cp.tile([128, 2 * NC], F32)
            for h in range(2):
                pt = ps_sm.tile([128, NC], F32, tag="pst")
                nc.tensor.transpose(
                    out=pt[:, :], in_=w_sb[:, h * 128:(h + 1) * 128],
                    identity=id_sb[:NC, :NC],
                )
                copy_out(wT[:, h * NC:(h + 1) * NC], pt[:, :])

            # P = protos @ protos.T / TAU, diag masked; lse over rows (symmetric)
            pP = ps_sm.tile([C, C], F32, tag="pst")
            for h in range(2):
                nc.tensor.matmul(
                    out=pP[:, :], lhsT=proT[:, h * C:(h + 1) * C],
                    rhs=proT[:, h * C:(h + 1) * C], start=(h == 0), stop=(h == 1),
                )
            P_sb = cp.tile([C, C], F32)
            # P/TAU - 1e9*I
            idbig = cp.tile([C, C], F32)
            nc.vector.tensor_scalar(
                out=idbig[:, :], in0=id_sb[:C, :C], scalar1=1e9, scalar2=None,
                op0=OP.mult,
            )
            nc.vector.tensor_scalar(
                out=P_sb[:, :], in0=pP[:, :], scalar1=1.0 / TAU, scalar2=None,
                op0=OP.mult,
            )
            nc.vector.tensor_tensor(out=P_sb[:, :], in0=P_sb[:, :], in1=idbig[:, :], op=OP.subtract)
            pmax = cp.tile([C, 1], F32)
            nc.vector.tensor_reduce(out=pmax[:, :], in_=P_sb[:, :], axis=mybir.AxisListType.X, op=OP.max)
            npmax = cp.tile([C, 1], F32)
            nc.vector.tensor_scalar(out=npmax[:, :], in0=pmax[:, :], scalar1=-1.0, scalar2=None, op0=OP.mult)
            pexp = cp.tile([C, C], F32)
            psum_col = cp.tile([C, 1], F32)
            nc.scalar.activation(pexp[:, :], P_sb[:, :], AF.Exp, bias=npmax[:, :1], scale=1.0, accum_out=psum_col[:, :1])
            plog = cp.tile([C, 1], F32)
            nc.scalar.activation(plog[:, :], psum_col[:, :], AF.Ln)
            lsePm_col = cp.tile([C, 1], F32)
            nc.vector.tensor_tensor(out=lsePm_col[:, :], in0=plog[:, :], in1=pmax[:, :], op=OP.add)


            # persistent accumulators (split so the sumexp AllReduce can fire early)
            acc = accp.tile([128, 90], F32)
            nc.vector.memset(acc[:, :], 0.0)
            acc2 = accp.tile([128, 3], F32)
            nc.vector.memset(acc2[:, :], 0.0)
            labf_all = accp.tile([128, BL * NMT], F32)
            posc_all = accp.tile([128, BL * NMT], F32)

            # ---------------- phase 1: per image ----------------
            for b in range(BL):
                # ---- loads ----
                obj_sb = bigp.tile([128, NQT * D], F32, tag="obj")
                nc.gpsimd.memset(obj_sb[:, 7 * D:], 0.0)
                nc.sync.dma_start(
                    out=obj_sb[:, :7 * D].rearrange("p (t d) -> p t d", d=D),
                    in_=obj_d.ap()[b, :7 * 128, :].rearrange("(t p) d -> p t d", p=128),
                )
                nc.sync.dma_start(out=obj_sb[:4, 7 * D:], in_=obj_d.ap()[b, 7 * 128:, :])

                idxc = smp.tile([128, NMT], I32, tag="idxc")
                nc.gpsimd.memset(idxc[:, :], 0)
                labc = smp.tile([128, NMT], I32, tag="labc")
                nc.gpsimd.memset(labc[:, :], 1 << 30)
                for m in range(NMT):
                    r = NROWS[m]
                    nc.sync.dma_start(
                        out=idxc[:r, m:m + 1],
                        in_=idx_d.ap()[b, m * 128:m * 128 + r].rearrange("(p o) -> p o", o=1),
                    )
                    nc.sync.dma_start(
                        out=labc[:r, m:m + 1],
                        in_=lab_d.ap()[b, m * 128:m * 128 + r].rearrange("(p o) -> p o", o=1),
                    )
                for m in range(NMT):
                    nc.vector.tensor_copy(labf_all[:, b * NMT + m: b * NMT + m + 1], labc[:, m:m + 1])

                # ---- matched gather (indices pre-adjusted by +b*900 host-side) ----
                matched = medp.tile([128, NMT * D], F32, tag="matched")
                for m in range(NMT):
                    r = NROWS[m]
                    nc.gpsimd.indirect_dma_start(
                        out=matched[:r, m * D:(m + 1) * D],
                        out_offset=None,
                        in_=obj_flat[:, :],
                        in_offset=bass.IndirectOffsetOnAxis(ap=idxc[:r, m:m + 1], axis=0),
                    )

                # ---- norms ----
                q2 = smp.tile([128, NQT], F32, tag="q2")
                for t in range(NQT):
                    jt = jkp.tile([128, D], F32, tag="j256")
                    nc.scalar.activation(
                        jt[:, :], obj_sb[:, t * D:(t + 1) * D], AF.Square,
                        accum_out=q2[:, t:t + 1],
                    )
                qn = smp.tile([128, NQT], F32, tag="qn")
                nc.scalar.activation(qn[:, :], q2[:, :], AF.Sqrt)
                nc.vector.tensor_scalar(out=qn[:, :], in0=qn[:, :], scalar1=1e-12, scalar2=None, op0=OP.max)
                rq = smp.tile([128, NQT], F32, tag="rq")
                nc.vector.reciprocal(rq[:, :], qn[:, :])

                # is_matched via compare with broadcast idx row
                # scatter 1.0 at matched query positions into zeroed DRAM, read back
                idxrc = smp.tile([128, NMT], I32, tag="idxrc")
                nc.gpsimd.memset(idxrc[:, :], NQT * 128 - 1)  # pads -> trash slot 1023
                for m in range(NMT):
                    r = NROWS[m]
                    nc.sync.dma_start(
                        out=idxrc[:r, m:m + 1],
                        in_=idxr_d.ap()[b, m * 128:m * 128 + r].rearrange("(p o) -> p o", o=1),
                    )
                zrow = smp.tile([1, NQT * 128], F32, tag="zrow")
                nc.vector.memset(zrow[:, :], 0.0)
                nc.sync.dma_start(
                    out=ismd[b].ap().rearrange("(o n) x -> o (n x)", o=1), in_=zrow[:, :])
                for m in range(NMT):
                    r = NROWS[m]
                    nc.gpsimd.indirect_dma_start(
                        out=ismd[b].ap()[:, :], out_offset=bass.IndirectOffsetOnAxis(
                            ap=idxrc[:r, m:m + 1], axis=0),
                        in_=ones_col[:r, :1], in_offset=None,
                    )
                ism = smp.tile([128, NQT], F32, tag="ism")
                nc.sync.dma_start(
                    out=ism[:, :],
                    in_=ismd[b].ap().rearrange("(t p) x -> p (t x)", p=128))
                # rqm = rq * (1 - ism)
                rqm = smp.tile([128, NQT], F32, tag="rqm")
                nc.vector.tensor_scalar(out=rqm[:, :], in0=ism[:, :], scalar1=-1.0, scalar2=1.0, op0=OP.mult, op1=OP.add)
                nc.vector.tensor_tensor(out=rqm[:, :], in0=rqm[:, :], in1=rq[:, :], op=OP.mult)

                objn = bigp.tile([128, NQT * D], F32R, tag="objn")
                for t in range(NQT):
                    nc.scalar.activation(
                        objn[:, t * D:(t + 1) * D], obj_sb[:, t * D:(t + 1) * D],
                        AF.Copy, scale=rqm[:, t:t + 1],
                    )

                # matched norms + normalize
                m2 = smp.tile([128, NMT], F32, tag="m2")
                nc.vector.memset(m2[:, :], 1.0)
                for m in range(NMT):
                    r = NROWS[m]
                    jt = jkp.tile([128, D], F32, tag="j256")
                    nc.scalar.activation(
                        jt[:r, :], matched[:r, m * D:(m + 1) * D], AF.Square,
                        accum_out=m2[:r, m:m + 1],
                    )
                mn = smp.tile([128, NMT], F32, tag="mn")
                nc.scalar.activation(mn[:, :], m2[:, :], AF.Sqrt)
                nc.vector.tensor_scalar(out=mn[:, :], in0=mn[:, :], scalar1=1e-12, scalar2=None, op0=OP.max)
                rm = smp.tile([128, NMT], F32, tag="rm")
                nc.vector.reciprocal(rm[:, :], mn[:, :])
                matched_n = medp.tile([128, NMT * D], F32, tag="matchedn")
                for m in range(NMT):
                    r = NROWS[m]
                    nc.scalar.activation(
                        matched_n[:r, m * D:(m + 1) * D], matched[:r, m * D:(m + 1) * D],
                        AF.Copy, scale=rm[:r, m:m + 1],
                    )

                # ---- transposes: matched_nT, matchedT [128, 600], objnT [128, 1800] ----
                mnT = medp.tile([128, 2 * Nm], F32, tag="mnT")
                mnT_r = medp.tile([128, 2 * Nm], F32R, tag="mnTr")
                for m in range(NMT):
                    r = NROWS[m]
                    for h in range(2):
                        pt = ps_sm.tile([128, 300], F32, tag="pst")
                        nc.tensor.transpose(
                            out=pt[:, :r],
                            in_=matched_n[:r, m * D + h * 128: m * D + (h + 1) * 128],
                            identity=id_sb[:r, :r],
                        )
                        copy_out(mnT[:, h * Nm + m * 128: h * Nm + m * 128 + r], pt[:, :r])
                        copy_out(mnT_r[:, h * Nm + m * 128: h * Nm + m * 128 + r], pt[:, :r])

                objnT = otp.tile([128, 2 * Q], F32R, tag="objnT")
                for t in range(NQT):
                    r = QROWS[t]
                    for h in range(2):
                        pt = ps_sm.tile([128, 300], F32R, tag="pst")
                        nc.tensor.transpose(
                            out=pt[:, :r],
                            in_=objn[:r, t * D + h * 128: t * D + (h + 1) * 128],
                            identity=id_sb_r[:r, :r],
                        )
                        copy_out(objnT[:, h * Q + t * 128: h * Q + t * 128 + r], pt[:, :r])

                # ---- sims = matched_n @ protos.T  [300, 90] ----
                sims_sb = medp.tile([128, NMT * C], F32, tag="sims")
                for m in range(NMT):
                    r = NROWS[m]
                    psim = ps_sm.tile([128, 300], F32, tag="pst")
                    for h in range(2):
                        nc.tensor.matmul(
                            out=psim[:r, :C],
                            lhsT=mnT[:, h * Nm + m * 128: h * Nm + m * 128 + r],
                            rhs=proT[:, h * C:(h + 1) * C],
                            start=(h == 0), stop=(h == 1),
                        )
                    copy_out(sims_sb[:r, m * C:(m + 1) * C], psim[:r, :C])

                # ---- pos, dists, CEC exp accumulation, rank-in-class ----
                dcol = smp.tile([128, NMT], F32, tag="dcol")
                for m in range(NMT):
                    r = NROWS[m]
                    mask = jkp.tile([128, C], F32, tag="mask")
                    nc.vector.tensor_scalar(
                        out=mask[:r, :], in0=io90[:r, :], scalar1=labf_all[:r, b * NMT + m: b * NMT + m + 1],
                        scalar2=None, op0=OP.is_equal,
                    )
                    j90 = jkp.tile([128, C], F32, tag="j90")
                    nc.gpsimd.tensor_tensor(out=j90[:r, :], in0=sims_sb[:r, m * C:(m + 1) * C], in1=mask[:r, :], op=OP.mult)
                    nc.vector.tensor_reduce(out=posc_all[:r, b * NMT + m: b * NMT + m + 1], in_=j90[:r, :], axis=mybir.AxisListType.X, op=OP.add)
                    nc.vector.tensor_scalar(
                        out=dcol[:r, m:m + 1], in0=posc_all[:r, b * NMT + m: b * NMT + m + 1],
                        scalar1=-1.0, scalar2=1.0, op0=OP.mult, op1=OP.add,
                    )
                    # expnet += exp(10*sims - 10) * (1 - mask)
                    expm = jkp.tile([128, C], F32, tag="expm")
                    nc.scalar.activation(expm[:r, :], sims_sb[:r, m * C:(m + 1) * C], AF.Exp, bias=nshift_col[:r, :1], scale=1.0 / TAU)
                    nm_ = jkp.tile([128, C], F32, tag="nm_")
                    nc.vector.tensor_scalar(out=nm_[:r, :], in0=mask[:r, :], scalar1=-1.0, scalar2=1.0, op0=OP.mult, op1=OP.add)
                    nc.vector.tensor_tensor(out=expm[:r, :], in0=expm[:r, :], in1=nm_[:r, :], op=OP.mult)
                    nc.vector.tensor_tensor(out=acc[:r, 0:C], in0=acc[:r, 0:C], in1=expm[:r, :], op=OP.add)

                selm = smp.tile([128, NMT], F32, tag="selm")
                d_bc = medp1.tile([128, Nm], F32, tag="dbc")
                lab_bc = medp1.tile([128, Nm], F32, tag="labbc")
                for m in range(NMT):
                    r = NROWS[m]
                    col_bcast(d_bc[:, m * 128: m * 128 + r], dcol[:, m:m + 1], r, id_sb)
                    col_bcast(lab_bc[:, m * 128: m * 128 + r],
                              labf_all[:, b * NMT + m: b * NMT + m + 1], r, id_sb)

                for m in range(NMT):
                    r = NROWS[m]
                    eq = jkp.tile([128, Nm], F32, tag="eq")
                    nc.vector.tensor_scalar(
                        out=eq[:r, :], in0=lab_bc[:r, :],
                        scalar1=labf_all[:r, b * NMT + m: b * NMT + m + 1], scalar2=None, op0=OP.is_equal,
                    )
                    gt = jkp.tile([128, Nm], F32, tag="gt")
                    nc.vector.tensor_scalar(
                        out=gt[:r, :], in0=d_bc[:r, :], scalar1=dcol[:r, m:m + 1],
                        scalar2=None, op0=OP.is_gt,
                    )
                    j300 = jkp.tile([128, Nm], F32, tag="j300b")
                    cnt = jkp.tile([128, 1], F32, tag="cnt")
                    nc.gpsimd.tensor_tensor(out=j300[:r, :], in0=eq[:r, :], in1=gt[:r, :], op=OP.mult)
                    nc.vector.tensor_reduce(out=cnt[:r, :1], in_=j300[:r, :], axis=mybir.AxisListType.X, op=OP.add)
                    nc.vector.tensor_scalar(out=selm[:r, m:m + 1], in0=cnt[:r, :], scalar1=4.5, scalar2=None, op0=OP.is_lt)

                # ---- simQ = matched_n @ obj_n.T  [300, 900] ----
                simq_sb = sqp.tile([128, NMT * Q], F32, tag="simq")
                for m in range(NMT):
                    r = NROWS[m]
                    psq = ps_sq.tile([128, Q], F32, tag="psq")
                    for c0, c1 in ((0, 512), (512, Q)):
                        for h in range(2):
                            nc.tensor.matmul(
                                out=psq[:r, c0:c1],
                                lhsT=mnT_r[:, h * Nm + m * 128: h * Nm + m * 128 + r],
                                rhs=objnT[:, h * Q + c0: h * Q + c1],
                                start=(h == 0), stop=(h == 1),
                            )
                    copy_out(simq_sb[:r, m * Q:(m + 1) * Q], psq[:r, :])

                # ---- top-5 threshold, multihot, wcnt ----
                mh = mhp.tile([128, NMT * Q], F32R, tag="mh")
                wcnt = smp.tile([128, NMT], F32, tag="wcnt")
                nc.vector.memset(wcnt[:, :], 0.0)
                thr = smp.tile([128, NMT], F32, tag="thr")
                for m in range(NMT):
                    r = NROWS[m]
                    mx8 = jkp.tile([128, 8], F32, tag="mx8")
                    nc.vector.max(out=mx8[:r, :], in_=simq_sb[:r, m * Q:(m + 1) * Q])
                    nc.vector.tensor_scalar(out=thr[:r, m:m + 1], in0=mx8[:r, 4:5], scalar1=1e-30, scalar2=None, op0=OP.max)
                    nc.vector.tensor_scalar(
                        out=mh[:r, m * Q:(m + 1) * Q], in0=simq_sb[:r, m * Q:(m + 1) * Q],
                        scalar1=thr[:r, m:m + 1], scalar2=None,
                        op0=OP.is_ge, op1=OP.add, accum_out=wcnt[:r, m:m + 1],
                    )

                # ---- multihot transpose [q, n] ----
                mhT = mhtp.tile([128, NQT * Nm], F32R, tag="mhT")
                for m in range(NMT):
                    r = NROWS[m]
                    for t in range(NQT):
                        qr = QROWS[t]
                        pt = ps_sm.tile([128, 300], F32R, tag="pst")
                        nc.tensor.transpose(
                            out=pt[:qr, :r],
                            in_=mh[:r, m * Q + t * 128: m * Q + t * 128 + qr],
                            identity=id_sb_r[:r, :r],
                        )
                        # scale rows by ||obj_q|| so that objn @ mhT_w == obj @ multihot.T
                        nc.vector.tensor_scalar(
                            out=mhT[:qr, t * Nm + m * 128: t * Nm + m * 128 + r],
                            in0=pt[:qr, :r], scalar1=qn[:qr, t:t + 1], scalar2=None,
                            op0=OP.mult,
                        )

                # ---- nbr sum: rawT = matchedT + obj.T @ multihot.T  [256 x 300] ----
                rawT = medp1.tile([128, 2 * Nm], F32, tag="rawT")
                for h in range(2):
                    pn = ps_nbr.tile([128, Nm], F32, tag="pnbr")
                    for t in range(NQT):
                        qr = QROWS[t]
                        nc.tensor.matmul(
                            out=pn[:, :],
                            lhsT=objn[:qr, t * D + h * 128: t * D + (h + 1) * 128],
                            rhs=mhT[:qr, t * Nm:(t + 1) * Nm],
                            start=(t == 0), stop=(t == NQT - 1),
                        )
                    # + matched.T via transpose-matmuls into the same accumulation
                    for m in range(NMT):
                        r = NROWS[m]
                        nc.tensor.matmul(
                            out=pn[:r if False else slice(None), :][:, m * 128: m * 128 + r] if False else pn[:, m * 128: m * 128 + r],
                            lhsT=matched[:r, m * D + h * 128: m * D + (h + 1) * 128],
                            rhs=id_sb[:r, :r],
                            is_transpose=True,
                            start=False, stop=(m == NMT - 1),
                            skip_group_check=True,
                        )
                    copy_out(rawT[:, h * Nm:(h + 1) * Nm], pn[:, :])

                # ---- logits & focal ----
                den = smp.tile([128, NMT], F32, tag="den")
                nc.vector.tensor_scalar(out=den[:, :], in0=wcnt[:, :], scalar1=1.0, scalar2=None, op0=OP.add)
                sden = smp.tile([128, NMT], F32, tag="sden")
                nc.vector.reciprocal(sden[:, :], den[:, :])

                fl = smp.tile([128, NMT], F32, tag="fl")
                hasn = smp.tile([128, NMT], F32, tag="hasn")
                nc.vector.tensor_scalar(out=hasn[:, :], in0=wcnt[:, :], scalar1=0.5, scalar2=None, op0=OP.is_gt)

                for m in range(NMT):
                    r = NROWS[m]
                    pl = ps_sm.tile([128, 300], F32, tag="pst")
                    for h in range(2):
                        nc.tensor.matmul(
                            out=pl[:r, :NC],
                            lhsT=rawT[:, h * Nm + m * 128: h * Nm + m * 128 + r],
                            rhs=wT[:, h * NC:(h + 1) * NC],
                            start=(h == 0), stop=False,
                        )
                    nc.tensor.matmul(
                        out=pl[:r, :NC], lhsT=ones_row[:1, :r], rhs=bcls_sb[:1, :],
                        start=False, stop=True,
                    )
                    lg = jkp.tile([128, NC], F32, tag="lg")
                    nc.vector.tensor_scalar(out=lg[:r, :], in0=pl[:r, :NC], scalar1=sden[:r, m:m + 1], scalar2=None, op0=OP.mult)
                    # focal with e1 = exp(-l) shared:
                    #   sig(l) = 1/(1+e1);  softplus(l) = l + ln(1+e1)
                    #   X_j = softplus(l)*sig(l)^2 for j<last
                    #   Y   = softplus(-l)*sig(-l)^2 = (ln(1+e1) - ... ) at last col
                    e1 = jkp.tile([128, NC], F32, tag="e1")
                    nc.scalar.activation(e1[:r, :], lg[:r, :], AF.Exp, scale=-1.0)
                    l1p = jkp.tile([128, NC], F32, tag="l1p")
                    nc.scalar.activation(l1p[:r, :], e1[:r, :], AF.Ln, bias=1.0, scale=1.0)
                    den1 = jkp.tile([128, NC], F32, tag="den1")
                    nc.vector.tensor_scalar(out=den1[:r, :], in0=e1[:r, :], scalar1=1.0, scalar2=None, op0=OP.add)
                    sg = jkp.tile([128, NC], F32, tag="sg")
                    nc.vector.reciprocal(sg[:r, :], den1[:r, :])
                    sp = jkp.tile([128, NC], F32, tag="sp")
                    nc.vector.tensor_tensor(out=sp[:r, :], in0=lg[:r, :], in1=l1p[:r, :], op=OP.add)
                    s2 = jkp.tile([128, NC], F32, tag="s2")
                    nc.vector.tensor_tensor(out=s2[:r, :], in0=sg[:r, :], in1=sg[:r, :], op=OP.mult)
                    X = jkp.tile([128, NC], F32, tag="X")
                    xs = jkp.tile([128, 1], F32, tag="xs")
                    nc.vector.tensor_tensor(out=X[:r, :], in0=s2[:r, :], in1=sp[:r, :], op=OP.mult)
                    nc.vector.tensor_reduce(out=xs[:r, :1], in_=X[:r, :], axis=mybir.AxisListType.X, op=OP.add)
                    # Y at last col: sig(-l) = e1/(1+e1) = e1*sg; softplus(-l) = ln(1+e1)
                    sgn = jkp.tile([128, 1], F32, tag="sgn")
                    nc.vector.tensor_tensor(out=sgn[:r, :], in0=e1[:r, NC - 1:NC], in1=sg[:r, NC - 1:NC], op=OP.mult)
                    Y = jkp.tile([128, 1], F32, tag="Y")
                    nc.vector.tensor_tensor(out=Y[:r, :], in0=sgn[:r, :], in1=sgn[:r, :], op=OP.mult)
                    nc.vector.tensor_tensor(out=Y[:r, :], in0=Y[:r, :], in1=l1p[:r, NC - 1:NC], op=OP.mult)
                    # fl = (0.75*(xs - X_last) + 0.25*Y)/NC
                    t1 = jkp.tile([128, 1], F32, tag="t1")
                    nc.vector.tensor_tensor(out=t1[:r, :], in0=xs[:r, :], in1=X[:r, NC - 1:NC], op=OP.subtract)
                    nc.vector.tensor_scalar(out=t1[:r, :], in0=t1[:r, :], scalar1=0.75 / NC, scalar2=None, op0=OP.mult)
                    nc.vector.tensor_scalar(out=Y[:r, :], in0=Y[:r, :], scalar1=0.25 / NC, scalar2=None, op0=OP.mult)
                    nc.vector.tensor_tensor(out=fl[:r, m:m + 1], in0=t1[:r, :], in1=Y[:r, :], op=OP.add)

                # ---- sul contributions (sel & has_nbr & fl) ----
                for m in range(NMT):
                    r = NROWS[m]
                    c1 = jkp.tile([128, 1], F32, tag="c1")
                    nc.vector.tensor_tensor(out=c1[:r, :], in0=selm[:r, m:m + 1], in1=hasn[:r, m:m + 1], op=OP.mult)
                    c2 = jkp.tile([128, 1], F32, tag="c2")
                    nc.vector.tensor_tensor(out=c2[:r, :], in0=c1[:r, :], in1=fl[:r, m:m + 1], op=OP.mult)
                    nc.vector.tensor_tensor(out=acc2[:r, 0:1], in0=acc2[:r, 0:1], in1=c2[:r, :], op=OP.add)
                    nc.vector.tensor_tensor(out=acc2[:r, 1:2], in0=acc2[:r, 1:2], in1=c1[:r, :], op=OP.add)

            # ---------------- AllReduce 1: expnet(90) only (fires early) ----------------
            pr1 = ps_sm.tile([1, 300], F32, tag="pst")
            nc.tensor.matmul(out=pr1[:1, :C], lhsT=ones_col[:, :1], rhs=acc[:, 0:C], start=True, stop=True)
            r1 = smp.tile([1, 96], F32, tag="r1")
            nc.vector.memset(r1[:, :], 0.0)
            nc.vector.tensor_copy(r1[:1, :C], pr1[:1, :C])
            nc.sync.dma_start(out=ar1_in.ap()[:, :], in_=r1[:, :])
            nc.gpsimd.collective_compute(
                "AllReduce", OP.add, replica_groups=groups,
                ins=[ar1_in.ap()[:, :]], outs=[ar1_out.ap()[:, :]],
            )
            g1 = smp.tile([96, 1], F32, tag="g1")
            nc.sync.dma_start(out=g1[:, :], in_=ar1_out.ap()[0, :].rearrange("(p o) -> p o", o=1))

            # lse_neg col = logaddexp(lsePm, SHIFT + ln(sumexp))
            lnS = smp.tile([C, 1], F32, tag="lnS")
            nc.scalar.activation(lnS[:, :], g1[:C, :], AF.Ln)
            nc.vector.tensor_scalar(out=lnS[:, :], in0=lnS[:, :], scalar1=SHIFT, scalar2=None, op0=OP.add)
            mx = smp.tile([C, 1], F32, tag="mx")
            nc.vector.tensor_tensor(out=mx[:, :], in0=lnS[:, :], in1=lsePm_col[:, :], op=OP.max)
            mnm = smp.tile([C, 1], F32, tag="mnm")
            nc.vector.tensor_tensor(out=mnm[:, :], in0=lnS[:, :], in1=lsePm_col[:, :], op=OP.min)
            nc.vector.tensor_tensor(out=mnm[:, :], in0=mnm[:, :], in1=mx[:, :], op=OP.subtract)
            ef = smp.tile([C, 1], F32, tag="ef")
            nc.scalar.activation(ef[:, :], mnm[:, :], AF.Exp)
            l1 = smp.tile([C, 1], F32, tag="l1")
            nc.scalar.activation(l1[:, :], ef[:, :], AF.Ln, bias=1.0, scale=1.0)
            lneg = smp.tile([C, 1], F32, tag="lneg")
            nc.vector.tensor_tensor(out=lneg[:, :], in0=mx[:, :], in1=l1[:, :], op=OP.add)
            ln_bc = medp1.tile([128, C], F32, tag="lnbc")
            col_bcast(ln_bc[:, :C], lneg[:, :1], C, id_sb)

            # ---------------- phase 2: per-sample CEC ----------------
            for b in range(BL):
                for m in range(NMT):
                    r = NROWS[m]
                    k = b * NMT + m
                    mask = jkp.tile([128, C], F32, tag="mask")
                    nc.vector.tensor_scalar(
                        out=mask[:r, :], in0=io90[:r, :], scalar1=labf_all[:r, k:k + 1],
                        scalar2=None, op0=OP.is_equal,
                    )
                    j90 = jkp.tile([128, C], F32, tag="j90")
                    lnn = jkp.tile([128, 1], F32, tag="lnn")
                    nc.vector.tensor_tensor(out=j90[:r, :], in0=ln_bc[:r, :], in1=mask[:r, :], op=OP.mult)
                    nc.vector.tensor_reduce(out=lnn[:r, :1], in_=j90[:r, :], axis=mybir.AxisListType.X, op=OP.add)
                    posS = jkp.tile([128, 1], F32, tag="posS")
                    nc.vector.tensor_scalar(out=posS[:r, :], in0=posc_all[:r, k:k + 1], scalar1=1.0 / TAU, scalar2=None, op0=OP.mult)
                    mxc = jkp.tile([128, 1], F32, tag="mxc")
                    nc.vector.tensor_tensor(out=mxc[:r, :], in0=posS[:r, :], in1=lnn[:r, :], op=OP.max)
                    mnc = jkp.tile([128, 1], F32, tag="mnc")
                    nc.vector.tensor_tensor(out=mnc[:r, :], in0=posS[:r, :], in1=lnn[:r, :], op=OP.min)
                    nc.vector.tensor_tensor(out=mnc[:r, :], in0=mnc[:r, :], in1=mxc[:r, :], op=OP.subtract)
                    efc = jkp.tile([128, 1], F32, tag="efc")
                    nc.scalar.activation(efc[:r, :], mnc[:r, :], AF.Exp)
                    l1c = jkp.tile([128, 1], F32, tag="l1c")
                    nc.scalar.activation(l1c[:r, :], efc[:r, :], AF.Ln, bias=1.0, scale=1.0)
                    nc.vector.tensor_tensor(out=mxc[:r, :], in0=mxc[:r, :], in1=l1c[:r, :], op=OP.add)
                    nc.vector.tensor_tensor(out=mxc[:r, :], in0=mxc[:r, :], in1=posS[:r, :], op=OP.subtract)
                    nc.vector.tensor_tensor(out=acc2[:r, 2:3], in0=acc2[:r, 2:3], in1=mxc[:r, :], op=OP.add)

            # ---------------- AllReduce 2: [sul_num, sul_cnt, cec_sum] ----------------
            pr2 = ps_sm.tile([1, 300], F32, tag="pst")
            nc.tensor.matmul(out=pr2[:1, :3], lhsT=ones_col[:, :1], rhs=acc2[:, 0:3], start=True, stop=True)
            r2 = smp.tile([1, 8], F32, tag="r2")
            nc.vector.memset(r2[:, :], 0.0)
            nc.vector.tensor_copy(r2[:1, :3], pr2[:1, :3])
            nc.sync.dma_start(out=ar2_in.ap()[:, :], in_=r2[:, :])
            nc.gpsimd.collective_compute(
                "AllReduce", OP.add, replica_groups=groups,
                ins=[ar2_in.ap()[:, :]], outs=[ar2_out.ap()[:, :]],
            )
            g2 = smp.tile([1, 8], F32, tag="g2")
            nc.sync.dma_start(out=g2[:, :], in_=ar2_out.ap()[:, :])

            # ---------------- final output ----------------
            outr = smp.tile([1, 2], F32, tag="outr")
            sulrow = smp.tile([1, 2], F32, tag="sulrow")
            nc.sync.dma_start(out=sulrow[:, :], in_=ar2_out.ap()[:, 0:2])
            denf = smp.tile([1, 1], F32, tag="denf")
            nc.vector.tensor_scalar(out=denf[:, :], in0=sulrow[:1, 1:2], scalar1=1.0, scalar2=None, op0=OP.max)
            rdf = smp.tile([1, 1], F32, tag="rdf")
            nc.vector.reciprocal(rdf[:, :], denf[:, :])
            nc.vector.tensor_tensor(out=outr[:1, 0:1], in0=sulrow[:1, 0:1], in1=rdf[:1, :], op=OP.mult)
            nc.vector.tensor_scalar(out=outr[:1, 1:2], in0=g2[:1, 2:3], scalar1=1.0 / (B * Nm), scalar2=None, op0=OP.mult)
            nc.sync.dma_start(out=out_d.ap().rearrange("(a b) -> a b", a=1), in_=outr[:, :])

    return nc


def make_in_maps(obj_embs, prototypes, W_cls, b_cls, match_src_idx, match_labels):
    identc = np.eye(128, dtype=np.float32)
    iota90c = np.tile(np.arange(C, dtype=np.float32), (128, 1))
    qiotac = (np.arange(128, dtype=np.float32)[:, None]
              + 128.0 * np.arange(NQT, dtype=np.float32)[None, :]).astype(np.float32)
    adj = (np.arange(BL, dtype=np.int32) * Q)[:, None]
    in_maps = []
    for c in range(NCORES):
        sl = slice(c * BL, (c + 1) * BL)
        in_maps.append({
            "obj": np.ascontiguousarray(obj_embs[sl]).astype(np.float32),
            "midx": (match_src_idx[sl].astype(np.int32) + adj),
            "midxraw": np.ascontiguousarray(match_src_idx[sl]).astype(np.int32),
            "mlab": np.ascontiguousarray(match_labels[sl]).astype(np.int32),
            "protos": np.ascontiguousarray(prototypes).astype(np.float32),
            "wcls": np.ascontiguousarray(W_cls).astype(np.float32),
            "bcls": np.ascontiguousarray(b_cls).astype(np.float32).reshape(1, NC),
            "identc": identc,
            "iota90c": iota90c,
            "qiotac": qiotac,
        })
    return in_maps


_CACHE = {}


def _install_ntff_shim():
    """Register the axon NTFF profile hook (test-time only; grading never traces)."""
    import types
    try:
        from antenv.axon_hooks import get_axon_ntff_profile_hook  # noqa: F401
        return
    except ImportError:
        pass
    import antenv
    from trn_agent_boot.trn_boot import _ntff_profile_via_ctypes
    mod = types.ModuleType("antenv.axon_hooks")
    _hook = [None]
    mod.set_axon_ntff_profile_hook = lambda h: _hook.__setitem__(0, h)
    mod.get_axon_ntff_profile_hook = lambda: _hook[0]
    sys.modules["antenv.axon_hooks"] = mod
    antenv.axon_hooks = mod
    mod.set_axon_ntff_profile_hook(_ntff_profile_via_ctypes("/opt/axon/libaxon_pjrt.so"))
    orig_upload = bass_utils.upload_artifacts
    def _safe_upload(tmpdir):
        try:
            return orig_upload(tmpdir)
        except Exception as e:
            print("upload_artifacts skipped:", e)
            return tmpdir
    bass_utils.upload_artifacts = _safe_upload


def kernel(obj_embs, prototypes, W_cls, b_cls, match_src_idx, match_labels,
           _trace=False, **extra):
    if _trace:
        _install_ntff_shim()
    if "nc" not in _CACHE:
        _CACHE["nc"] = build_nc()
    nc = _CACHE["nc"]
    in_maps = make_in_maps(obj_embs, prototypes, W_cls, b_cls,
                           match_src_idx, match_labels)
    res = bass_utils.run_bass_kernel_spmd(
        nc, in_maps, core_ids=list(range(NCORES)), trace=_trace,
    )
    _CACHE["last_results"] = res
    return np.asarray(res.results[0]["out"], dtype=np.float32).reshape(2)


if __name__ == "__main__":
    nc = build_nc()
    print("built ok")
